# revision 1
# baseline (speedup 1.0000x reference)
"""Trainium2 Bass kernel for nn_Attention_53077205844237 (GNN edge softmax).

Computation (reference):
    q   = x_j + e_ij                          # [E, 128]
    w   = tanh(concat([q, x_i], -1) @ W + b)  # [E, 8]
    out = segment_softmax(w, e_row)           # [E, 8], segments = rows

Problem structure (hardcoded): E = 131072 edges, IN = 128, F = 8,
N = 4096 nodes, and e_row = repeat(arange(4096), 32) -- every segment is a
contiguous, 32-edge block.  The segment softmax is therefore a softmax over
fixed 32-edge groups.  Since |tanh| < 1, exp() cannot overflow and the
segment-max subtraction is mathematically a no-op -- only a segment *sum*
is needed.

Sharding: edges are split contiguously across the 8 NeuronCores
(16384 edges = 512 whole segments per core), so the softmax is fully local
to each core: no collectives and no index tensors on device.

Device layout: inputs are passed feature-major (x^T, [128, E/8] per core) so
the contraction (over features) sits on the SBUF partition dim and the PE
matmul needs no on-device transposes.  The three float32r matmuls per
512-edge chunk accumulate W1^T@x_j^T + W1^T@e_ij^T + W2^T@x_i^T in one PSUM
bank (the q = x_j + e_ij add is free via accumulation); a 2048-edge compute
batch fills 4 banks at partitions 0..7.  ACT applies tanh(+bias) then exp;
DVE does the 32-wide segment reduce_sum, reciprocal, and the broadcast
multiply; output is stored as out^T [8, E/8] and de-transposed on host.

Two kernel builders exist: a TileContext version (_build_bass) and a raw
bacc pipeline with manual semaphores (_build_bass_raw, default) that skips
Tile's exit barrier.  Input loads stream on both HWDGE rings (SP + ACT),
output stores ride SWDGE (gpsimd); per-ring-slot DMA semaphores keep at most
one outstanding transfer per semaphore so `>= 16` waits are sound.  The load
plan tapers only the tail (512/512/512/256/128/128) so the dependency chain
after the last input bytes is short while the DMA rings stay saturated
during ramp-up.  Measured on 8 axon-tunneled TRN2 cores: ~96 us mean /
~103 us max per-core NEFF exec, rel err ~8e-5 vs the f32 reference
(~86 us of that is the contended HBM streaming floor at ~300 GB/s/core,
plus ~12 us fixed NRT preamble/postamble).
"""

import sys
import types
from contextlib import ExitStack

if "/opt/trn_rl_repo" not in sys.path:
    sys.path.insert(0, "/opt/trn_rl_repo")

import numpy as np

# ---------------------------------------------------------------------------
# Optional NTFF-profile hook (used only when _run(trace=True); harmless else).
# The container's antenv package lacks axon_hooks; provide it so
# run_bass_kernel_spmd's trace path can find the profiler hook.
# ---------------------------------------------------------------------------
if "antenv.axon_hooks" not in sys.modules:
    _hooks_mod = types.ModuleType("antenv.axon_hooks")
    _hook_box = [None]
    _hooks_mod.set_axon_ntff_profile_hook = lambda h: _hook_box.__setitem__(0, h)
    _hooks_mod.get_axon_ntff_profile_hook = lambda: _hook_box[0]
    sys.modules["antenv.axon_hooks"] = _hooks_mod
    try:
        from trn_agent_boot.trn_boot import _ntff_profile_via_ctypes

        _hooks_mod.set_axon_ntff_profile_hook(
            _ntff_profile_via_ctypes("/opt/axon/libaxon_pjrt.so")
        )
    except Exception:
        pass

# Problem constants (hardcoded per the task contract).
E = 131072
IN = 128
F = 8
N_NODES = 4096
DEG = 32
N_CORES = 8
ES = E // N_CORES          # edges per core = 16384
LD = 2048                  # input DMA batch (edges): 1 MB per tensor per load
ST = 2048                  # compute batch (edges) = half of PSUM (4 banks)
CH = 512                   # matmul moving free dim / PSUM bank chunk
GROUPS = ST // CH          # chunks per compute batch = 4

_COMPILED = None           # cached (nc) bass module


def _build_bass():
    import concourse.bacc as bacc
    import concourse.tile as tile
    from concourse import mybir

    f32 = mybir.dt.float32
    f32r = mybir.dt.float32r
    AF = mybir.ActivationFunctionType

    nc = bacc.Bacc("TRN2", target_bir_lowering=False, debug=False,
                   num_devices=N_CORES)

    xjT = nc.dram_tensor("xjT", [IN, ES], f32r, kind="ExternalInput")
    eijT = nc.dram_tensor("eijT", [IN, ES], f32r, kind="ExternalInput")
    xiT = nc.dram_tensor("xiT", [IN, ES], f32r, kind="ExternalInput")
    w1 = nc.dram_tensor("W1", [IN, F], f32r, kind="ExternalInput")
    w2 = nc.dram_tensor("W2", [IN, F], f32r, kind="ExternalInput")
    bv = nc.dram_tensor("b", [F, 1], f32, kind="ExternalInput")
    outT = nc.dram_tensor("outT", [F, ES], f32, kind="ExternalOutput")

    loads = _load_plan()

    with tile.TileContext(nc) as tc:
        with (
            tc.tile_pool(name="consts", bufs=1) as consts,
            tc.tile_pool(name="ins", bufs=4) as ins_pool,
            tc.tile_pool(name="work", bufs=3) as work,
            tc.tile_pool(name="psum", bufs=2, space="PSUM") as psum_pool,
            tc.tile_pool(name="outp", bufs=3) as outp,
        ):
            w1_t = consts.tile([IN, F], f32r)
            nc.sync.dma_start(out=w1_t[:], in_=w1[:])
            w2_t = consts.tile([IN, F], f32r)
            nc.sync.dma_start(out=w2_t[:], in_=w2[:])
            bias_t = consts.tile([F, 1], f32)
            nc.sync.dma_start(out=bias_t[:], in_=bv[:])

            for li, (lpos, lsize) in enumerate(loads):
                lsl = slice(lpos, lpos + lsize)
                # Spread input loads over both HWDGE rings (SP + ACT).
                xi_eng = nc.sync if li % 2 == 0 else nc.scalar
                xj_t = ins_pool.tile([IN, lsize], f32r, tag="xj")
                nc.sync.dma_start(out=xj_t[:], in_=xjT[:, lsl])
                eij_t = ins_pool.tile([IN, lsize], f32r, tag="eij")
                nc.scalar.dma_start(out=eij_t[:], in_=eijT[:, lsl])
                xi_t = ins_pool.tile([IN, lsize], f32r, tag="xi")
                xi_eng.dma_start(out=xi_t[:], in_=xiT[:, lsl])

                for bpos in range(0, lsize, ST):
                    size = min(ST, lsize - bpos)
                    nseg = size // DEG
                    osl = slice(lpos + bpos, lpos + bpos + size)

                    # One 512-edge chunk per PSUM bank; partitions 0..7 = f.
                    ps_full = psum_pool.tile([F, ST], f32, tag="ps")
                    ps = ps_full[:, 0:size]
                    for cpos in range(0, size, CH):
                        cw = min(CH, size - cpos)
                        csl = slice(bpos + cpos, bpos + cpos + cw)
                        po = ps[:, cpos:cpos + cw]
                        nc.tensor.matmul(po, w1_t[:], xj_t[:, csl],
                                         start=True, stop=False)
                        nc.tensor.matmul(po, w1_t[:], eij_t[:, csl],
                                         start=False, stop=False)
                        nc.tensor.matmul(po, w2_t[:], xi_t[:, csl],
                                         start=False, stop=True)

                    # ew = exp(tanh(psum + b)); |tanh| < 1, no max needed.
                    wt = work.tile([F, size], f32, tag="w")
                    nc.scalar.activation(out=wt[:], in_=ps[:], func=AF.Tanh,
                                         bias=bias_t[:, 0:1])
                    ew = work.tile([F, size], f32, tag="ew")
                    nc.scalar.activation(out=ew[:], in_=wt[:], func=AF.Exp)

                    # Segment sums over each 32-edge block, then reciprocal.
                    denom = work.tile([F, nseg], f32, tag="denom")
                    nc.vector.reduce_sum(
                        out=denom[:],
                        in_=ew[:].rearrange("p (n d) -> p n d", d=DEG),
                        axis=mybir.AxisListType.X,
                    )
                    recip = work.tile([F, nseg], f32, tag="recip")
                    nc.vector.reciprocal(out=recip[:], in_=denom[:])

                    ot = outp.tile([F, size], f32, tag="o")
                    nc.vector.tensor_mul(
                        out=ot[:].rearrange("p (n d) -> p n d", d=DEG),
                        in0=ew[:].rearrange("p (n d) -> p n d", d=DEG),
                        in1=recip[:].unsqueeze(-1).broadcast_to(
                            [F, nseg, DEG]),
                    )
                    nc.sync.dma_start(out=outT[:, osl], in_=ot[:])

    nc.compile()
    return nc


def _load_plan():
    # Full-size loads up front keep the DMA rings saturated during pipeline
    # ramp (DMA is the bottleneck engine); taper only the tail so the final
    # dependency chain after the last input bytes is short.
    tail = [CH, CH, CH, CH // 2, CH // 4, CH // 4]
    loads = []
    pos = 0
    while pos < ES - sum(tail):
        loads.append((pos, LD))
        pos += LD
    for sz in tail:
        loads.append((pos, sz))
        pos += sz
    assert pos == ES, (pos, ES)
    return loads


def _build_bass_raw():
    """Raw bacc pipeline (no TileContext): manual semaphores, no exit
    butterfly barrier.  Engine roles: SP = xj/xi-even input DMAs,
    ACT = eij/xi-odd input DMAs + tanh + exp, PE = matmuls,
    DVE = reduce/recip/mul, GPSIMD = output stores (SWDGE) + final
    semaphore clear (re-execution safety)."""
    import concourse.bacc as bacc
    import concourse.bass as bass
    from concourse import mybir

    f32 = mybir.dt.float32
    f32r = mybir.dt.float32r
    AF = mybir.ActivationFunctionType

    nc = bacc.Bacc("TRN2", target_bir_lowering=False, debug=False,
                   num_devices=N_CORES)

    xjT = nc.dram_tensor("xjT", [IN, ES], f32r, kind="ExternalInput")
    eijT = nc.dram_tensor("eijT", [IN, ES], f32r, kind="ExternalInput")
    xiT = nc.dram_tensor("xiT", [IN, ES], f32r, kind="ExternalInput")
    w1 = nc.dram_tensor("W1", [IN, F], f32r, kind="ExternalInput")
    w2 = nc.dram_tensor("W2", [IN, F], f32r, kind="ExternalInput")
    bv = nc.dram_tensor("b", [F, 1], f32, kind="ExternalInput")
    outT = nc.dram_tensor("outT", [F, ES], f32, kind="ExternalOutput")

    loads = _load_plan()
    NB = len(loads)
    NIN = 5      # input ring slots per tensor
    NWK = 3      # work/out ring slots
    NEW = NWK    # ew ring slots

    with ExitStack() as ctx:
        # Per-ring-slot DMA semaphores: at most ONE outstanding transfer per
        # semaphore, so a `>= 16` wait really means "that transfer landed"
        # (increments from concurrent transfers on one semaphore interleave).
        all_sems = []

        def mksem(name):
            s = ctx.enter_context(nc.semaphore(name))
            all_sems.append(s)
            return s

        s_xj = [mksem(f"s_xj{r}") for r in range(NIN)]
        s_eij = [mksem(f"s_eij{r}") for r in range(NIN)]
        s_xi = [mksem(f"s_xi{r}") for r in range(NIN)]
        s_out = [mksem(f"s_out{r}") for r in range(NWK)]
        s_mm = mksem("s_mm")
        s_red = mksem("s_red")
        s_rcp = mksem("s_rcp")
        s_psf = mksem("s_psf")
        s_exp = mksem("s_exp")
        s_mul = mksem("s_mul")
        s_const = mksem("s_const")

        in_xj = [ctx.enter_context(nc.sbuf_tensor(f"in_xj{r}", [IN, LD], f32r))
                 for r in range(NIN)]
        in_eij = [ctx.enter_context(nc.sbuf_tensor(f"in_eij{r}", [IN, LD], f32r))
                  for r in range(NIN)]
        in_xi = [ctx.enter_context(nc.sbuf_tensor(f"in_xi{r}", [IN, LD], f32r))
                 for r in range(NIN)]
        w_t = [ctx.enter_context(nc.sbuf_tensor(f"w{r}", [F, LD], f32))
               for r in range(NWK)]
        ew_t = [ctx.enter_context(nc.sbuf_tensor(f"ew{r}", [F, LD], f32))
                for r in range(NEW)]
        o_t = [ctx.enter_context(nc.sbuf_tensor(f"o{r}", [F, LD], f32))
               for r in range(NWK)]
        dn_t = ctx.enter_context(nc.sbuf_tensor("dn", [F, LD // DEG], f32))
        rc_t = ctx.enter_context(nc.sbuf_tensor("rc", [F, LD // DEG], f32))
        ps_t = [ctx.enter_context(nc.psum_tensor(f"ps{r}", [F, LD], f32))
                for r in range(2)]
        w1_t = ctx.enter_context(nc.sbuf_tensor("w1s", [IN, F], f32r))
        w2_t = ctx.enter_context(nc.sbuf_tensor("w2s", [IN, F], f32r))
        b_t = ctx.enter_context(nc.sbuf_tensor("bs", [F, 1], f32))

        with nc.Block() as block:

            @block.sync
            def _(sp):
                for b, (pos, size) in enumerate(loads):
                    sl = slice(pos, pos + size)
                    if b >= NIN:
                        sp.wait_ge(s_mm, b - (NIN - 1))
                    sp.dma_start(out=in_xj[b % NIN][:, 0:size],
                                 in_=xjT[:, sl]).then_inc(s_xj[b % NIN], 16)
                    if b % 2 == 0:
                        sp.dma_start(out=in_xi[b % NIN][:, 0:size],
                                     in_=xiT[:, sl]).then_inc(s_xi[b % NIN], 16)

            @block.scalar
            def _(act):
                for b, (pos, size) in enumerate(loads):
                    sl = slice(pos, pos + size)
                    if b >= NIN:
                        act.wait_ge(s_mm, b - (NIN - 1))
                    act.dma_start(out=in_eij[b % NIN][:, 0:size],
                                  in_=eijT[:, sl]).then_inc(s_eij[b % NIN], 16)
                    if b % 2 == 1:
                        act.dma_start(out=in_xi[b % NIN][:, 0:size],
                                      in_=xiT[:, sl]).then_inc(s_xi[b % NIN], 16)
                    if b >= 2:
                        bb = b - 2
                        bsz = loads[bb][1]
                        if bb == 0:
                            act.wait_ge(s_const, 48)
                        act.wait_ge(s_mm, bb + 1)
                        if bb >= NEW:
                            act.wait_ge(s_mul, bb - (NEW - 1))
                        act.activation(
                            out=w_t[bb % NWK][:, 0:bsz],
                            in_=ps_t[bb % 2][:, 0:bsz],
                            func=AF.Tanh, bias=b_t[:, 0:1],
                        ).then_inc(s_psf, 1)
                        if SAFE_INTRA:
                            act.wait_ge(s_psf, bb + 1)
                        act.activation(
                            out=ew_t[bb % NEW][:, 0:bsz],
                            in_=w_t[bb % NWK][:, 0:bsz],
                            func=AF.Exp,
                        ).then_inc(s_exp, 1)
                for bb in (NB - 2, NB - 1):
                    bsz = loads[bb][1]
                    act.wait_ge(s_mm, bb + 1)
                    if bb >= NEW:
                        act.wait_ge(s_mul, bb - (NEW - 1))
                    act.activation(
                        out=w_t[bb % NWK][:, 0:bsz],
                        in_=ps_t[bb % 2][:, 0:bsz],
                        func=AF.Tanh, bias=b_t[:, 0:1],
                    ).then_inc(s_psf, 1)
                    if SAFE_INTRA:
                        act.wait_ge(s_psf, bb + 1)
                    act.activation(
                        out=ew_t[bb % NEW][:, 0:bsz],
                        in_=w_t[bb % NWK][:, 0:bsz],
                        func=AF.Exp,
                    ).then_inc(s_exp, 1)

            @block.tensor
            def _(pe):
                pe.wait_ge(s_const, 48)
                for b, (pos, size) in enumerate(loads):
                    r = b % NIN
                    n_use = b // NIN + 1
                    pe.wait_ge(s_xj[r], 16 * n_use)
                    pe.wait_ge(s_eij[r], 16 * n_use)
                    pe.wait_ge(s_xi[r], 16 * n_use)
                    if b >= 2:
                        pe.wait_ge(s_psf, b - 1)
                    ps = ps_t[b % 2]
                    nch = (size + CH - 1) // CH
                    for c in range(nch):
                        cw = min(CH, size - c * CH)
                        csl = slice(c * CH, c * CH + cw)
                        last = pe.matmul(ps[:, csl],
                                         w1_t[:], in_xj[b % NIN][:, csl],
                                         start=True, stop=False)
                        pe.matmul(ps[:, csl],
                                  w1_t[:], in_eij[b % NIN][:, csl],
                                  start=False, stop=False)
                        last = pe.matmul(ps[:, csl],
                                         w2_t[:], in_xi[b % NIN][:, csl],
                                         start=False, stop=True)
                    last.then_inc(s_mm, 1)

            @block.vector
            def _(dve):
                for b, (pos, size) in enumerate(loads):
                    nseg = size // DEG
                    dve.wait_ge(s_exp, b + 1)
                    ew = ew_t[b % NEW]
                    dve.reduce_sum(
                        out=dn_t[:, 0:nseg],
                        in_=ew[:, 0:size].rearrange("p (n d) -> p n d", d=DEG),
                        axis=mybir.AxisListType.X,
                    ).then_inc(s_red, 1)
                    if SAFE_INTRA:
                        dve.wait_ge(s_red, b + 1)
                    dve.reciprocal(
                        out=rc_t[:, 0:nseg], in_=dn_t[:, 0:nseg]
                    ).then_inc(s_rcp, 1)
                    if SAFE_INTRA:
                        dve.wait_ge(s_rcp, b + 1)
                    if b >= NWK:
                        dve.wait_ge(s_out[b % NWK], 16 * ((b - NWK) // NWK + 1))
                    dve.tensor_mul(
                        out=o_t[b % NWK][:, 0:size].rearrange(
                            "p (n d) -> p n d", d=DEG),
                        in0=ew[:, 0:size].rearrange("p (n d) -> p n d", d=DEG),
                        in1=rc_t[:, 0:nseg].unsqueeze(-1).broadcast_to(
                            [F, nseg, DEG]),
                    ).then_inc(s_mul, 1)

            @block.gpsimd
            def _(gp):
                gp.dma_start(out=w1_t[:], in_=w1[:]).then_inc(s_const, 16)
                gp.dma_start(out=w2_t[:], in_=w2[:]).then_inc(s_const, 16)
                gp.dma_start(out=b_t[:], in_=bv[:]).then_inc(s_const, 16)
                for b, (pos, size) in enumerate(loads):
                    sl = slice(pos, pos + size)
                    gp.wait_ge(s_mul, b + 1)
                    gp.dma_start(
                        out=outT[:, sl],
                        in_=o_t[b % NWK][:, 0:size],
                    ).then_inc(s_out[b % NWK], 16)
                # Ensure output stores have landed before this stream ends.
                # No in-kernel semaphore clear: the NRT postamble performs
                # sync_barrier + sema_reset + dma_rearm between executions.
                for r in range(NWK):
                    n_r = len(range(r, NB, NWK))
                    gp.wait_ge(s_out[r], 16 * n_r)

    nc.compile()
    return nc


def _build_bass_raw2():
    """Like _build_bass_raw, but the three input tensors are packed on host
    into ONE block-major HBM buffer per core: block b = contiguous
    [128, 3*size] region (xj | eij | xi side by side).  Each load is a single
    fully-sequential HBM read (13 transfers instead of 39, one DMA semaphore
    per ring slot)."""
    import concourse.bacc as bacc
    from concourse import mybir

    f32 = mybir.dt.float32
    f32r = mybir.dt.float32r
    AF = mybir.ActivationFunctionType

    nc = bacc.Bacc("TRN2", target_bir_lowering=False, debug=False,
                   num_devices=N_CORES)

    pk = nc.dram_tensor("pk", [IN * 3 * ES], f32r, kind="ExternalInput")
    w1 = nc.dram_tensor("W1", [IN, F], f32r, kind="ExternalInput")
    w2 = nc.dram_tensor("W2", [IN, F], f32r, kind="ExternalInput")
    bv = nc.dram_tensor("b", [F, 1], f32, kind="ExternalInput")
    outT = nc.dram_tensor("outT", [F, ES], f32, kind="ExternalOutput")

    loads = _load_plan()
    NB = len(loads)
    NIN = 5      # input ring slots
    NWK = 3      # work/out ring slots

    with ExitStack() as ctx:
        all_sems = []

        def mksem(name):
            s = ctx.enter_context(nc.semaphore(name))
            all_sems.append(s)
            return s

        s_in = [mksem(f"s_in{r}") for r in range(NIN)]
        s_out = [mksem(f"s_out{r}") for r in range(NWK)]
        s_mm = mksem("s_mm")
        s_red = mksem("s_red")
        s_rcp = mksem("s_rcp")
        s_psf = mksem("s_psf")
        s_exp = mksem("s_exp")
        s_mul = mksem("s_mul")
        s_const = mksem("s_const")

        in_t = [ctx.enter_context(nc.sbuf_tensor(f"in{r}", [IN, 3 * LD], f32r))
                for r in range(NIN)]
        w_t = [ctx.enter_context(nc.sbuf_tensor(f"w{r}", [F, LD], f32))
               for r in range(NWK)]
        ew_t = [ctx.enter_context(nc.sbuf_tensor(f"ew{r}", [F, LD], f32))
                for r in range(NWK)]
        o_t = [ctx.enter_context(nc.sbuf_tensor(f"o{r}", [F, LD], f32))
               for r in range(NWK)]
        dn_t = ctx.enter_context(nc.sbuf_tensor("dn", [F, LD // DEG], f32))
        rc_t = ctx.enter_context(nc.sbuf_tensor("rc", [F, LD // DEG], f32))
        ps_t = [ctx.enter_context(nc.psum_tensor(f"ps{r}", [F, LD], f32))
                for r in range(2)]
        w1_t = ctx.enter_context(nc.sbuf_tensor("w1s", [IN, F], f32r))
        w2_t = ctx.enter_context(nc.sbuf_tensor("w2s", [IN, F], f32r))
        b_t = ctx.enter_context(nc.sbuf_tensor("bs", [F, 1], f32))

        def pk_view(pos, size):
            off = IN * 3 * pos
            return pk[off:off + IN * 3 * size].rearrange("(p c) -> p c", p=IN)

        with nc.Block() as block:

            @block.sync
            def _(sp):
                for b, (pos, size) in enumerate(loads):
                    if b % 2 != 0:
                        continue
                    if b >= NIN:
                        sp.wait_ge(s_mm, b - (NIN - 1))
                    sp.dma_start(out=in_t[b % NIN][:, 0:3 * size],
                                 in_=pk_view(pos, size)).then_inc(
                                     s_in[b % NIN], 16)

            @block.scalar
            def _(act):
                for b, (pos, size) in enumerate(loads):
                    if b % 2 == 1:
                        if b >= NIN:
                            act.wait_ge(s_mm, b - (NIN - 1))
                        act.dma_start(out=in_t[b % NIN][:, 0:3 * size],
                                      in_=pk_view(pos, size)).then_inc(
                                          s_in[b % NIN], 16)
                    if b >= 2:
                        bb = b - 2
                        bsz = loads[bb][1]
                        if bb == 0:
                            act.wait_ge(s_const, 48)
                        act.wait_ge(s_mm, bb + 1)
                        if bb >= NWK:
                            act.wait_ge(s_mul, bb - (NWK - 1))
                        act.activation(
                            out=w_t[bb % NWK][:, 0:bsz],
                            in_=ps_t[bb % 2][:, 0:bsz],
                            func=AF.Tanh, bias=b_t[:, 0:1],
                        ).then_inc(s_psf, 1)
                        act.wait_ge(s_psf, bb + 1)
                        act.activation(
                            out=ew_t[bb % NWK][:, 0:bsz],
                            in_=w_t[bb % NWK][:, 0:bsz],
                            func=AF.Exp,
                        ).then_inc(s_exp, 1)
                for bb in (NB - 2, NB - 1):
                    bsz = loads[bb][1]
                    act.wait_ge(s_mm, bb + 1)
                    if bb >= NWK:
                        act.wait_ge(s_mul, bb - (NWK - 1))
                    act.activation(
                        out=w_t[bb % NWK][:, 0:bsz],
                        in_=ps_t[bb % 2][:, 0:bsz],
                        func=AF.Tanh, bias=b_t[:, 0:1],
                    ).then_inc(s_psf, 1)
                    act.wait_ge(s_psf, bb + 1)
                    act.activation(
                        out=ew_t[bb % NWK][:, 0:bsz],
                        in_=w_t[bb % NWK][:, 0:bsz],
                        func=AF.Exp,
                    ).then_inc(s_exp, 1)

            @block.tensor
            def _(pe):
                pe.wait_ge(s_const, 48)
                for b, (pos, size) in enumerate(loads):
                    r = b % NIN
                    pe.wait_ge(s_in[r], 16 * (b // NIN + 1))
                    if b >= 2:
                        pe.wait_ge(s_psf, b - 1)
                    ps = ps_t[b % 2]
                    it = in_t[r]
                    nch = (size + CH - 1) // CH
                    for c in range(nch):
                        cw = min(CH, size - c * CH)
                        cp = c * CH
                        last = pe.matmul(ps[:, cp:cp + cw], w1_t[:],
                                         it[:, cp:cp + cw],
                                         start=True, stop=False)
                        pe.matmul(ps[:, cp:cp + cw], w1_t[:],
                                  it[:, size + cp:size + cp + cw],
                                  start=False, stop=False)
                        last = pe.matmul(ps[:, cp:cp + cw], w2_t[:],
                                         it[:, 2 * size + cp:2 * size + cp + cw],
                                         start=False, stop=True)
                    last.then_inc(s_mm, 1)

            @block.vector
            def _(dve):
                for b, (pos, size) in enumerate(loads):
                    nseg = size // DEG
                    dve.wait_ge(s_exp, b + 1)
                    ew = ew_t[b % NWK]
                    dve.reduce_sum(
                        out=dn_t[:, 0:nseg],
                        in_=ew[:, 0:size].rearrange("p (n d) -> p n d", d=DEG),
                        axis=mybir.AxisListType.X,
                    ).then_inc(s_red, 1)
                    dve.wait_ge(s_red, b + 1)
                    dve.reciprocal(
                        out=rc_t[:, 0:nseg], in_=dn_t[:, 0:nseg]
                    ).then_inc(s_rcp, 1)
                    dve.wait_ge(s_rcp, b + 1)
                    if b >= NWK:
                        dve.wait_ge(s_out[b % NWK], 16 * ((b - NWK) // NWK + 1))
                    dve.tensor_mul(
                        out=o_t[b % NWK][:, 0:size].rearrange(
                            "p (n d) -> p n d", d=DEG),
                        in0=ew[:, 0:size].rearrange("p (n d) -> p n d", d=DEG),
                        in1=rc_t[:, 0:nseg].unsqueeze(-1).broadcast_to(
                            [F, nseg, DEG]),
                    ).then_inc(s_mul, 1)

            @block.gpsimd
            def _(gp):
                gp.dma_start(out=w1_t[:], in_=w1[:]).then_inc(s_const, 16)
                gp.dma_start(out=w2_t[:], in_=w2[:]).then_inc(s_const, 16)
                gp.dma_start(out=b_t[:], in_=bv[:]).then_inc(s_const, 16)
                for b, (pos, size) in enumerate(loads):
                    sl = slice(pos, pos + size)
                    gp.wait_ge(s_mul, b + 1)
                    gp.dma_start(
                        out=outT[:, sl],
                        in_=o_t[b % NWK][:, 0:size],
                    ).then_inc(s_out[b % NWK], 16)
                for r in range(NWK):
                    n_r = len(range(r, NB, NWK))
                    gp.wait_ge(s_out[r], 16 * n_r)

    nc.compile()
    return nc


USE_RAW = True
PACKED = False      # packed single-buffer loses ~11us: concurrent
                    # per-tensor streams on separate queues beat one
                    # sequential 3MB stream (measured A/B, 8 reps)
SAFE_INTRA = True   # same-engine RAW sem waits (walrus emits DRAINs anyway)


def _get_compiled():
    global _COMPILED
    if _COMPILED is None:
        if USE_RAW:
            _COMPILED = _build_bass_raw2() if PACKED else _build_bass_raw()
        else:
            _COMPILED = _build_bass()
    return _COMPILED


def _pack_core_inputs(xjT_c, eijT_c, xiT_c):
    """Assemble the block-major packed buffer: for each load block,
    [128, 3*size] = (xj | eij | xi) columns, blocks back to back."""
    buf = np.empty(IN * 3 * ES, dtype=np.float32)
    off = 0
    for pos, size in _load_plan():
        n = IN * 3 * size
        seg = buf[off:off + n].reshape(IN, 3 * size)
        seg[:, 0:size] = xjT_c[:, pos:pos + size]
        seg[:, size:2 * size] = eijT_c[:, pos:pos + size]
        seg[:, 2 * size:3 * size] = xiT_c[:, pos:pos + size]
        off += n
    return buf


def _run_device(x_i, x_j, e_ij, W, b, trace=False, tmpdir=None,
                trace_cores=None):
    from concourse.bass_utils import run_bass_kernel_spmd

    nc = _get_compiled()

    W = np.ascontiguousarray(np.asarray(W, dtype=np.float32))
    b = np.asarray(b, dtype=np.float32).reshape(F, 1)
    W1 = np.ascontiguousarray(W[:IN])
    W2 = np.ascontiguousarray(W[IN:])

    in_maps = []
    for c in range(N_CORES):
        sl = slice(c * ES, (c + 1) * ES)
        xjT_c = np.ascontiguousarray(np.asarray(x_j[sl]).T)
        eijT_c = np.ascontiguousarray(np.asarray(e_ij[sl]).T)
        xiT_c = np.ascontiguousarray(np.asarray(x_i[sl]).T)
        if USE_RAW and PACKED:
            in_maps.append({
                "pk": _pack_core_inputs(xjT_c, eijT_c, xiT_c),
                "W1": W1,
                "W2": W2,
                "b": b,
            })
        else:
            in_maps.append({
                "xjT": xjT_c,
                "eijT": eijT_c,
                "xiT": xiT_c,
                "W1": W1,
                "W2": W2,
                "b": b,
            })

    kwargs = {}
    if trace:
        kwargs.update(trace=True,
                      trace_cores=(trace_cores if trace_cores is not None
                                   else list(range(N_CORES))),
                      tmpdir=tmpdir)
    res = run_bass_kernel_spmd(nc, in_maps, core_ids=list(range(N_CORES)),
                               **kwargs)

    out = np.empty((E, F), dtype=np.float32)
    for c in range(N_CORES):
        out[c * ES:(c + 1) * ES] = np.asarray(res.results[c]["outT"]).T
    return out, res


def _numpy_fallback(x_i, x_j, e_ij, adj, e_row, W, b):
    """Correct for arbitrary e_row (matches the reference semantics)."""
    x_i = np.asarray(x_i, np.float32)
    x_j = np.asarray(x_j, np.float32)
    e_ij = np.asarray(e_ij, np.float32)
    W = np.asarray(W, np.float32)
    b = np.asarray(b, np.float32)
    e_row = np.asarray(e_row).astype(np.int64)
    n = np.asarray(adj).shape[0]
    q = x_j + e_ij
    z = q @ W[:q.shape[1]] + x_i @ W[q.shape[1]:] + b
    w = np.tanh(z)
    m = np.full((n, w.shape[1]), -9e15, np.float32)
    np.maximum.at(m, e_row, w)
    ew = np.exp(w - m[e_row])
    denom = np.zeros((n, w.shape[1]), np.float32)
    np.add.at(denom, e_row, ew)
    return (ew / denom[e_row]).astype(np.float32)


def _is_fast_path(x_i, x_j, e_ij, adj, e_row, W, b):
    try:
        if np.asarray(x_i).shape != (E, IN):
            return False
        if np.asarray(x_j).shape != (E, IN):
            return False
        if np.asarray(e_ij).shape != (E, IN):
            return False
        if np.asarray(W).shape != (2 * IN, F):
            return False
        if np.asarray(b).reshape(-1).shape != (F,):
            return False
        if np.asarray(adj).shape[0] != N_NODES:
            return False
        er = np.asarray(e_row).reshape(-1)
        if er.shape != (E,):
            return False
        expected = np.repeat(np.arange(N_NODES, dtype=np.int64), DEG)
        return bool(np.array_equal(er.astype(np.int64), expected))
    except Exception:
        return False


def kernel(x_i, x_j, e_ij, adj, e_row, e_col, W, b, **_unused):
    if _is_fast_path(x_i, x_j, e_ij, adj, e_row, W, b):
        try:
            out, _ = _run_device(x_i, x_j, e_ij, W, b)
            return out
        except Exception as e:  # fail safe: correct > fast
            print(f"kernel: device path failed ({type(e).__name__}: {e}); "
                  "using numpy fallback", file=sys.stderr)
    return _numpy_fallback(x_i, x_j, e_ij, adj, e_row, W, b)



# revision 2
# speedup vs baseline: 1.6244x; 1.6244x over previous
"""Trainium2 Bass kernel for nn_Attention_53077205844237 (GNN edge softmax).

Computation (reference):
    q   = x_j + e_ij                          # [E, 128]
    w   = tanh(concat([q, x_i], -1) @ W + b)  # [E, 8]
    out = segment_softmax(w, e_row)           # [E, 8], segments = rows

Problem structure (hardcoded): E = 131072 edges, IN = 128, F = 8,
N = 4096 nodes, and e_row = repeat(arange(4096), 32) -- every segment is a
contiguous, 32-edge block.  Since |tanh| < 1, exp() cannot overflow and the
segment-max subtraction is mathematically a no-op -- only a segment *sum*
is needed.  Edges split contiguously across 8 NeuronCores (16384 = 512
whole segments per core): softmax fully local, no collectives.

Fast path ("fp8"): inputs are quantized host-side to fp8 e3m4 (1.8% RMS
element error; the problem tolerance is 2e-2 relative overall) with error
feedback pairing x_j/e_ij: e_ij is quantized AFTER absorbing x_j's
quantization error, so q = x_j + e_ij carries only a single quantization
error.  Per 2048-edge load: DVE adds q = xj + eij (fp8 -> bf16); the PE
runs 2 matmuls per 512-edge chunk (bf16 stationary x bf16/fp8 moving) into
ONE PSUM bank with the 4 chunks partition-stacked at bases {0,32,64,96}
via matmul tile_position, so ACT (tanh, exp) and DVE (32-wide segment sum,
reciprocal, broadcast mul) each run one instruction per bank across 128
partitions instead of 8.  Output is stored bank-stacked ([128, 512] incl.
24 garbage partitions per 32-group) and unshuffled on host.

Fallback ("raw"): the previous all-f32r 3-matmul pipeline (~105 us).
Final fallback: numpy (correct for arbitrary e_row).
"""

import sys
import types
from contextlib import ExitStack

if "/opt/trn_rl_repo" not in sys.path:
    sys.path.insert(0, "/opt/trn_rl_repo")

import numpy as np

# ---------------------------------------------------------------------------
# Optional NTFF-profile hook (used only when _run(trace=True); harmless else).
# ---------------------------------------------------------------------------
if "antenv.axon_hooks" not in sys.modules:
    _hooks_mod = types.ModuleType("antenv.axon_hooks")
    _hook_box = [None]
    _hooks_mod.set_axon_ntff_profile_hook = lambda h: _hook_box.__setitem__(0, h)
    _hooks_mod.get_axon_ntff_profile_hook = lambda: _hook_box[0]
    sys.modules["antenv.axon_hooks"] = _hooks_mod
    try:
        from trn_agent_boot.trn_boot import _ntff_profile_via_ctypes

        _hooks_mod.set_axon_ntff_profile_hook(
            _ntff_profile_via_ctypes("/opt/axon/libaxon_pjrt.so")
        )
    except Exception:
        pass

# Problem constants (hardcoded per the task contract).
E = 131072
IN = 128
F = 8
N_NODES = 4096
DEG = 32
N_CORES = 8
ES = E // N_CORES          # edges per core = 16384

# fp8 pipeline geometry
LD = 2048                  # edges per load == edges per PSUM bank
NB = ES // LD              # loads (= banks) per core = 8
CH = 512                   # psum chunk (col-tile) width
NCT = LD // CH             # chunks per bank = 4
NIN = 4                    # input ring slots
NQ = 3                     # q ring slots
NEW = 3                    # ew ring slots
NWK = 3                    # out ring slots

SAFE_INTRA = True          # same-engine RAW sem waits (walrus emits DRAINs)

_COMPILED = {}             # mode -> compiled bass module


def _build_fp8():
    import concourse.bacc as bacc
    from concourse import mybir

    f32 = mybir.dt.float32
    bf16 = mybir.dt.bfloat16
    f8 = mybir.dt.float8e3
    AF = mybir.ActivationFunctionType

    nc = bacc.Bacc("TRN2", target_bir_lowering=False, debug=False,
                   num_devices=N_CORES)

    xjT = nc.dram_tensor("xjT", [IN, ES], f8, kind="ExternalInput")
    eijT = nc.dram_tensor("eijT", [IN, ES], f8, kind="ExternalInput")
    xiT = nc.dram_tensor("xiT", [IN, ES], f8, kind="ExternalInput")
    w1 = nc.dram_tensor("W1", [IN, F], bf16, kind="ExternalInput")
    w2 = nc.dram_tensor("W2", [IN, F], bf16, kind="ExternalInput")
    bv = nc.dram_tensor("b", [IN, 1], f32, kind="ExternalInput")  # tiled bias
    # Stacked output: out2[32*c + f, CH*b + e] = out[edge LD*b + CH*c + e, f].
    # Partitions 8..31 of each 32-group are garbage (never read by host).
    out2 = nc.dram_tensor("out2", [IN, NB * CH], f32, kind="ExternalOutput")

    with ExitStack() as ctx:
        def mksem(name):
            return ctx.enter_context(nc.semaphore(name))

        s_xj = [mksem(f"s_xj{r}") for r in range(NIN)]
        s_eij = [mksem(f"s_eij{r}") for r in range(NIN)]
        s_xi = [mksem(f"s_xi{r}") for r in range(NIN)]
        s_out = [mksem(f"s_out{r}") for r in range(NWK)]
        s_q = mksem("s_q")
        s_mm = mksem("s_mm")
        s_psf = mksem("s_psf")
        s_exp = mksem("s_exp")
        s_red = mksem("s_red")
        s_rcp = mksem("s_rcp")
        s_mul = mksem("s_mul")
        s_const = mksem("s_const")

        in_xj = [ctx.enter_context(nc.sbuf_tensor(f"in_xj{r}", [IN, LD], f8))
                 for r in range(NIN)]
        in_eij = [ctx.enter_context(nc.sbuf_tensor(f"in_eij{r}", [IN, LD], f8))
                  for r in range(NIN)]
        in_xi = [ctx.enter_context(nc.sbuf_tensor(f"in_xi{r}", [IN, LD], f8))
                 for r in range(NIN)]
        q_t = [ctx.enter_context(nc.sbuf_tensor(f"q{r}", [IN, LD], bf16))
               for r in range(NQ)]
        w_t = [ctx.enter_context(nc.sbuf_tensor(f"w{r}", [IN, CH], f32))
               for r in range(2)]
        ew_t = [ctx.enter_context(nc.sbuf_tensor(f"ew{r}", [IN, CH], bf16))
                for r in range(NEW)]
        o_t = [ctx.enter_context(nc.sbuf_tensor(f"o{r}", [IN, CH], f32))
               for r in range(NWK)]
        dn_t = ctx.enter_context(nc.sbuf_tensor("dn", [IN, CH // DEG], bf16))
        rc_t = ctx.enter_context(nc.sbuf_tensor("rc", [IN, CH // DEG], bf16))
        ps_t = [ctx.enter_context(nc.psum_tensor(f"ps{r}", [IN, CH], f32))
                for r in range(NB)]
        w1_t = ctx.enter_context(nc.sbuf_tensor("w1s", [IN, F], bf16))
        w2_t = ctx.enter_context(nc.sbuf_tensor("w2s", [IN, F], bf16))
        b_t = ctx.enter_context(nc.sbuf_tensor("bs", [IN, 1], f32))

        def ld_slice(b):
            return slice(b * LD, (b + 1) * LD)

        with nc.Block() as block:

            @block.sync
            def _(sp):
                for b in range(NB):
                    r = b % NIN
                    if b >= NIN:
                        sp.wait_ge(s_q, b - NIN + 1)
                    sp.dma_start(out=in_xj[r][:], in_=xjT[:, ld_slice(b)]
                                 ).then_inc(s_xj[r], 16)
                    if b >= NIN:
                        sp.wait_ge(s_mm, b - NIN + 1)
                    sp.dma_start(out=in_xi[r][:], in_=xiT[:, ld_slice(b)]
                                 ).then_inc(s_xi[r], 16)

            @block.scalar
            def _(act):
                def act_bank(bb):
                    act.wait_ge(s_mm, bb + 1)
                    act.activation(
                        out=w_t[bb % 2][:], in_=ps_t[bb][:],
                        func=AF.Tanh, bias=b_t[:, 0:1],
                    ).then_inc(s_psf, 1)
                    if SAFE_INTRA:
                        act.wait_ge(s_psf, bb + 1)
                    if bb >= NEW:
                        act.wait_ge(s_mul, bb - NEW + 1)
                    act.activation(
                        out=ew_t[bb % NEW][:], in_=w_t[bb % 2][:],
                        func=AF.Exp,
                    ).then_inc(s_exp, 1)

                LAG = 2
                for b in range(NB):
                    r = b % NIN
                    if b >= NIN:
                        act.wait_ge(s_q, b - NIN + 1)
                    act.dma_start(out=in_eij[r][:], in_=eijT[:, ld_slice(b)]
                                  ).then_inc(s_eij[r], 16)
                    if b == 0:
                        act.wait_ge(s_const, 48)
                    if b >= LAG:
                        act_bank(b - LAG)
                for bb in range(NB - LAG, NB):
                    act_bank(bb)

            @block.tensor
            def _(pe):
                pe.wait_ge(s_const, 32)
                for b in range(NB):
                    r = b % NIN
                    pe.wait_ge(s_q, b + 1)
                    pe.wait_ge(s_xi[r], 16 * (b // NIN + 1))
                    ps = ps_t[b]
                    for c in range(NCT):
                        po = ps[32 * c:32 * c + F, :]
                        csl = slice(c * CH, (c + 1) * CH)
                        pe.matmul(po, w1_t[:], q_t[b % NQ][:, csl],
                                  start=True, stop=False,
                                  tile_position=(0, 32 * c))
                        last = pe.matmul(po, w2_t[:], in_xi[r][:, csl],
                                         start=False, stop=True,
                                         tile_position=(0, 32 * c))
                    last.then_inc(s_mm, 1)

            @block.vector
            def _(dve):
                def dve_bank(bb):
                    dve.wait_ge(s_exp, bb + 1)
                    ew = ew_t[bb % NEW]
                    with nc.allow_low_precision(
                            "32-wide segment sum of exp(tanh) in bf16; "
                            "error budget allows ~0.4%"):
                        dve.reduce_sum(
                            out=dn_t[:],
                            in_=ew[:].rearrange("p (n d) -> p n d", d=DEG),
                            axis=mybir.AxisListType.X,
                        ).then_inc(s_red, 1)
                        if SAFE_INTRA:
                            dve.wait_ge(s_red, bb + 1)
                        dve.reciprocal(out=rc_t[:], in_=dn_t[:]
                                       ).then_inc(s_rcp, 1)
                    if SAFE_INTRA:
                        dve.wait_ge(s_rcp, bb + 1)
                    if bb >= NWK:
                        dve.wait_ge(s_out[bb % NWK],
                                    16 * ((bb - NWK) // NWK + 1))
                    nseg = CH // DEG
                    dve.tensor_mul(
                        out=o_t[bb % NWK][:].rearrange("p (n d) -> p n d",
                                                       d=DEG),
                        in0=ew[:].rearrange("p (n d) -> p n d", d=DEG),
                        in1=rc_t[:].unsqueeze(-1).broadcast_to(
                            [IN, nseg, DEG]),
                    ).then_inc(s_mul, 1)

                LAGV = 1
                for b in range(NB):
                    r = b % NIN
                    n_use = 16 * (b // NIN + 1)
                    dve.wait_ge(s_xj[r], n_use)
                    dve.wait_ge(s_eij[r], n_use)
                    if b >= NQ:
                        dve.wait_ge(s_mm, b - NQ + 1)
                    dve.tensor_add(out=q_t[b % NQ][:], in0=in_xj[r][:],
                                   in1=in_eij[r][:]).then_inc(s_q, 1)
                    if b >= LAGV:
                        dve_bank(b - LAGV)
                for bb in range(NB - LAGV, NB):
                    dve_bank(bb)

            @block.gpsimd
            def _(gp):
                gp.dma_start(out=w1_t[:], in_=w1[:]).then_inc(s_const, 16)
                gp.dma_start(out=w2_t[:], in_=w2[:]).then_inc(s_const, 16)
                gp.dma_start(out=b_t[:], in_=bv[:]).then_inc(s_const, 16)
                for b in range(NB):
                    gp.wait_ge(s_mul, b + 1)
                    gp.dma_start(out=out2[:, b * CH:(b + 1) * CH],
                                 in_=o_t[b % NWK][:]
                                 ).then_inc(s_out[b % NWK], 16)
                for r in range(NWK):
                    n_r = len(range(r, NB, NWK))
                    gp.wait_ge(s_out[r], 16 * n_r)

    nc.compile()
    return nc


# ---------------------------------------------------------------------------
# Fallback: previous all-f32r raw pipeline (measured ~105 us end-to-end).
# ---------------------------------------------------------------------------
LDR = 2048                 # raw-path input DMA batch (edges)
CHR = 512                  # raw-path matmul chunk


def _load_plan_raw():
    tail = [CHR, CHR, CHR, CHR // 2, CHR // 4, CHR // 4]
    loads = []
    pos = 0
    while pos < ES - sum(tail):
        loads.append((pos, LDR))
        pos += LDR
    for sz in tail:
        loads.append((pos, sz))
        pos += sz
    assert pos == ES, (pos, ES)
    return loads


def _build_bass_raw():
    import concourse.bacc as bacc
    from concourse import mybir

    f32 = mybir.dt.float32
    f32r = mybir.dt.float32r
    AF = mybir.ActivationFunctionType

    nc = bacc.Bacc("TRN2", target_bir_lowering=False, debug=False,
                   num_devices=N_CORES)

    xjT = nc.dram_tensor("xjT", [IN, ES], f32r, kind="ExternalInput")
    eijT = nc.dram_tensor("eijT", [IN, ES], f32r, kind="ExternalInput")
    xiT = nc.dram_tensor("xiT", [IN, ES], f32r, kind="ExternalInput")
    w1 = nc.dram_tensor("W1", [IN, F], f32r, kind="ExternalInput")
    w2 = nc.dram_tensor("W2", [IN, F], f32r, kind="ExternalInput")
    bv = nc.dram_tensor("b", [F, 1], f32, kind="ExternalInput")
    outT = nc.dram_tensor("outT", [F, ES], f32, kind="ExternalOutput")

    loads = _load_plan_raw()
    NB_ = len(loads)
    NIN_ = 5
    NWK_ = 3
    NEW_ = NWK_

    with ExitStack() as ctx:
        def mksem(name):
            return ctx.enter_context(nc.semaphore(name))

        s_xj = [mksem(f"s_xj{r}") for r in range(NIN_)]
        s_eij = [mksem(f"s_eij{r}") for r in range(NIN_)]
        s_xi = [mksem(f"s_xi{r}") for r in range(NIN_)]
        s_out = [mksem(f"s_out{r}") for r in range(NWK_)]
        s_mm = mksem("s_mm")
        s_red = mksem("s_red")
        s_rcp = mksem("s_rcp")
        s_psf = mksem("s_psf")
        s_exp = mksem("s_exp")
        s_mul = mksem("s_mul")
        s_const = mksem("s_const")

        in_xj = [ctx.enter_context(nc.sbuf_tensor(f"in_xj{r}", [IN, LDR], f32r))
                 for r in range(NIN_)]
        in_eij = [ctx.enter_context(nc.sbuf_tensor(f"in_eij{r}", [IN, LDR], f32r))
                  for r in range(NIN_)]
        in_xi = [ctx.enter_context(nc.sbuf_tensor(f"in_xi{r}", [IN, LDR], f32r))
                 for r in range(NIN_)]
        w_t = [ctx.enter_context(nc.sbuf_tensor(f"w{r}", [F, LDR], f32))
               for r in range(NWK_)]
        ew_t = [ctx.enter_context(nc.sbuf_tensor(f"ew{r}", [F, LDR], f32))
                for r in range(NEW_)]
        o_t = [ctx.enter_context(nc.sbuf_tensor(f"o{r}", [F, LDR], f32))
               for r in range(NWK_)]
        dn_t = ctx.enter_context(nc.sbuf_tensor("dn", [F, LDR // DEG], f32))
        rc_t = ctx.enter_context(nc.sbuf_tensor("rc", [F, LDR // DEG], f32))
        ps_t = [ctx.enter_context(nc.psum_tensor(f"ps{r}", [F, LDR], f32))
                for r in range(2)]
        w1_t = ctx.enter_context(nc.sbuf_tensor("w1s", [IN, F], f32r))
        w2_t = ctx.enter_context(nc.sbuf_tensor("w2s", [IN, F], f32r))
        b_t = ctx.enter_context(nc.sbuf_tensor("bs", [F, 1], f32))

        with nc.Block() as block:

            @block.sync
            def _(sp):
                for b, (pos, size) in enumerate(loads):
                    sl = slice(pos, pos + size)
                    if b >= NIN_:
                        sp.wait_ge(s_mm, b - (NIN_ - 1))
                    sp.dma_start(out=in_xj[b % NIN_][:, 0:size],
                                 in_=xjT[:, sl]).then_inc(s_xj[b % NIN_], 16)
                    if b % 2 == 0:
                        sp.dma_start(out=in_xi[b % NIN_][:, 0:size],
                                     in_=xiT[:, sl]).then_inc(s_xi[b % NIN_], 16)

            @block.scalar
            def _(act):
                def act_tail(bb):
                    bsz = loads[bb][1]
                    act.wait_ge(s_mm, bb + 1)
                    if bb >= NEW_:
                        act.wait_ge(s_mul, bb - (NEW_ - 1))
                    act.activation(
                        out=w_t[bb % NWK_][:, 0:bsz],
                        in_=ps_t[bb % 2][:, 0:bsz],
                        func=AF.Tanh, bias=b_t[:, 0:1],
                    ).then_inc(s_psf, 1)
                    if SAFE_INTRA:
                        act.wait_ge(s_psf, bb + 1)
                    act.activation(
                        out=ew_t[bb % NEW_][:, 0:bsz],
                        in_=w_t[bb % NWK_][:, 0:bsz],
                        func=AF.Exp,
                    ).then_inc(s_exp, 1)

                for b, (pos, size) in enumerate(loads):
                    sl = slice(pos, pos + size)
                    if b >= NIN_:
                        act.wait_ge(s_mm, b - (NIN_ - 1))
                    act.dma_start(out=in_eij[b % NIN_][:, 0:size],
                                  in_=eijT[:, sl]).then_inc(s_eij[b % NIN_], 16)
                    if b % 2 == 1:
                        act.dma_start(out=in_xi[b % NIN_][:, 0:size],
                                      in_=xiT[:, sl]).then_inc(s_xi[b % NIN_], 16)
                    if b >= 2:
                        bb = b - 2
                        if bb == 0:
                            act.wait_ge(s_const, 48)
                        act_tail(bb)
                for bb in (NB_ - 2, NB_ - 1):
                    act_tail(bb)

            @block.tensor
            def _(pe):
                pe.wait_ge(s_const, 48)
                for b, (pos, size) in enumerate(loads):
                    r = b % NIN_
                    n_use = b // NIN_ + 1
                    pe.wait_ge(s_xj[r], 16 * n_use)
                    pe.wait_ge(s_eij[r], 16 * n_use)
                    pe.wait_ge(s_xi[r], 16 * n_use)
                    if b >= 2:
                        pe.wait_ge(s_psf, b - 1)
                    ps = ps_t[b % 2]
                    nch = (size + CHR - 1) // CHR
                    for c in range(nch):
                        cw = min(CHR, size - c * CHR)
                        csl = slice(c * CHR, c * CHR + cw)
                        pe.matmul(ps[:, csl],
                                  w1_t[:], in_xj[b % NIN_][:, csl],
                                  start=True, stop=False)
                        pe.matmul(ps[:, csl],
                                  w1_t[:], in_eij[b % NIN_][:, csl],
                                  start=False, stop=False)
                        last = pe.matmul(ps[:, csl],
                                         w2_t[:], in_xi[b % NIN_][:, csl],
                                         start=False, stop=True)
                    last.then_inc(s_mm, 1)

            @block.vector
            def _(dve):
                for b, (pos, size) in enumerate(loads):
                    nseg = size // DEG
                    dve.wait_ge(s_exp, b + 1)
                    ew = ew_t[b % NEW_]
                    dve.reduce_sum(
                        out=dn_t[:, 0:nseg],
                        in_=ew[:, 0:size].rearrange("p (n d) -> p n d", d=DEG),
                        axis=mybir.AxisListType.X,
                    ).then_inc(s_red, 1)
                    if SAFE_INTRA:
                        dve.wait_ge(s_red, b + 1)
                    dve.reciprocal(
                        out=rc_t[:, 0:nseg], in_=dn_t[:, 0:nseg]
                    ).then_inc(s_rcp, 1)
                    if SAFE_INTRA:
                        dve.wait_ge(s_rcp, b + 1)
                    if b >= NWK_:
                        dve.wait_ge(s_out[b % NWK_], 16 * ((b - NWK_) // NWK_ + 1))
                    dve.tensor_mul(
                        out=o_t[b % NWK_][:, 0:size].rearrange(
                            "p (n d) -> p n d", d=DEG),
                        in0=ew[:, 0:size].rearrange("p (n d) -> p n d", d=DEG),
                        in1=rc_t[:, 0:nseg].unsqueeze(-1).broadcast_to(
                            [F, nseg, DEG]),
                    ).then_inc(s_mul, 1)

            @block.gpsimd
            def _(gp):
                gp.dma_start(out=w1_t[:], in_=w1[:]).then_inc(s_const, 16)
                gp.dma_start(out=w2_t[:], in_=w2[:]).then_inc(s_const, 16)
                gp.dma_start(out=b_t[:], in_=bv[:]).then_inc(s_const, 16)
                for b, (pos, size) in enumerate(loads):
                    sl = slice(pos, pos + size)
                    gp.wait_ge(s_mul, b + 1)
                    gp.dma_start(
                        out=outT[:, sl],
                        in_=o_t[b % NWK_][:, 0:size],
                    ).then_inc(s_out[b % NWK_], 16)
                for r in range(NWK_):
                    n_r = len(range(r, NB_, NWK_))
                    gp.wait_ge(s_out[r], 16 * n_r)

    nc.compile()
    return nc


def _get_compiled(mode):
    if mode not in _COMPILED:
        _COMPILED[mode] = _build_fp8() if mode == "fp8" else _build_bass_raw()
    return _COMPILED[mode]


def _prep_inputs_fp8(x_i, x_j, e_ij, W, b):
    import ml_dtypes

    F8 = ml_dtypes.float8_e3m4
    BF16 = ml_dtypes.bfloat16

    W = np.asarray(W, dtype=np.float32)
    W1 = np.ascontiguousarray(W[:IN]).astype(BF16)
    W2 = np.ascontiguousarray(W[IN:]).astype(BF16)
    bias = np.asarray(b, dtype=np.float32).reshape(F)
    btile = np.zeros((IN, 1), np.float32)
    for cc in range(NCT):
        btile[32 * cc:32 * cc + F, 0] = bias

    in_maps = []
    for c in range(N_CORES):
        sl = slice(c * ES, (c + 1) * ES)
        xjT = np.ascontiguousarray(np.asarray(x_j[sl], np.float32).T)
        eijT = np.ascontiguousarray(np.asarray(e_ij[sl], np.float32).T)
        xiT = np.ascontiguousarray(np.asarray(x_i[sl], np.float32).T)
        xj8 = xjT.astype(F8)
        # error feedback: fold xj's quantization error into eij before its
        # quantization, so q = xj + eij carries a single quantization error
        eij8 = (eijT + (xjT - xj8.astype(np.float32))).astype(F8)
        xi8 = xiT.astype(F8)
        in_maps.append({
            "xjT": xj8,
            "eijT": eij8,
            "xiT": xi8,
            "W1": W1,
            "W2": W2,
            "b": btile,
        })
    return in_maps


def _gather_fp8(res):
    out = np.empty((E, F), dtype=np.float32)
    for c in range(N_CORES):
        o2 = np.asarray(res.results[c]["out2"], np.float32)  # [128, NB*CH]
        o4 = o2.reshape(NCT, 32, NB, CH)[:, :F]              # [c, f, b, e]
        out[c * ES:(c + 1) * ES] = o4.transpose(2, 0, 3, 1).reshape(ES, F)
    return out


def _prep_inputs_raw(x_i, x_j, e_ij, W, b):
    W = np.ascontiguousarray(np.asarray(W, dtype=np.float32))
    bias = np.asarray(b, dtype=np.float32).reshape(F, 1)
    W1 = np.ascontiguousarray(W[:IN])
    W2 = np.ascontiguousarray(W[IN:])
    in_maps = []
    for c in range(N_CORES):
        sl = slice(c * ES, (c + 1) * ES)
        in_maps.append({
            "xjT": np.ascontiguousarray(np.asarray(x_j[sl]).T),
            "eijT": np.ascontiguousarray(np.asarray(e_ij[sl]).T),
            "xiT": np.ascontiguousarray(np.asarray(x_i[sl]).T),
            "W1": W1,
            "W2": W2,
            "b": bias,
        })
    return in_maps


def _gather_raw(res):
    out = np.empty((E, F), dtype=np.float32)
    for c in range(N_CORES):
        out[c * ES:(c + 1) * ES] = np.asarray(res.results[c]["outT"]).T
    return out


def _run_device(x_i, x_j, e_ij, W, b, trace=False, tmpdir=None,
                trace_cores=None, mode="fp8"):
    from concourse.bass_utils import run_bass_kernel_spmd

    nc = _get_compiled(mode)
    if mode == "fp8":
        in_maps = _prep_inputs_fp8(x_i, x_j, e_ij, W, b)
    else:
        in_maps = _prep_inputs_raw(x_i, x_j, e_ij, W, b)

    kwargs = {}
    if trace:
        kwargs.update(trace=True,
                      trace_cores=(trace_cores if trace_cores is not None
                                   else list(range(N_CORES))),
                      tmpdir=tmpdir)
    res = run_bass_kernel_spmd(nc, in_maps, core_ids=list(range(N_CORES)),
                               **kwargs)

    out = _gather_fp8(res) if mode == "fp8" else _gather_raw(res)
    return out, res


def _numpy_fallback(x_i, x_j, e_ij, adj, e_row, W, b):
    """Correct for arbitrary e_row (matches the reference semantics)."""
    x_i = np.asarray(x_i, np.float32)
    x_j = np.asarray(x_j, np.float32)
    e_ij = np.asarray(e_ij, np.float32)
    W = np.asarray(W, np.float32)
    b = np.asarray(b, np.float32)
    e_row = np.asarray(e_row).astype(np.int64)
    n = np.asarray(adj).shape[0]
    q = x_j + e_ij
    z = q @ W[:q.shape[1]] + x_i @ W[q.shape[1]:] + b
    w = np.tanh(z)
    m = np.full((n, w.shape[1]), -9e15, np.float32)
    np.maximum.at(m, e_row, w)
    ew = np.exp(w - m[e_row])
    denom = np.zeros((n, w.shape[1]), np.float32)
    np.add.at(denom, e_row, ew)
    return (ew / denom[e_row]).astype(np.float32)


def _is_fast_path(x_i, x_j, e_ij, adj, e_row, W, b):
    try:
        if np.asarray(x_i).shape != (E, IN):
            return False
        if np.asarray(x_j).shape != (E, IN):
            return False
        if np.asarray(e_ij).shape != (E, IN):
            return False
        if np.asarray(W).shape != (2 * IN, F):
            return False
        if np.asarray(b).reshape(-1).shape != (F,):
            return False
        if np.asarray(adj).shape[0] != N_NODES:
            return False
        er = np.asarray(e_row).reshape(-1)
        if er.shape != (E,):
            return False
        expected = np.repeat(np.arange(N_NODES, dtype=np.int64), DEG)
        return bool(np.array_equal(er.astype(np.int64), expected))
    except Exception:
        return False


def kernel(x_i, x_j, e_ij, adj, e_row, e_col, W, b, **_unused):
    if _is_fast_path(x_i, x_j, e_ij, adj, e_row, W, b):
        for mode in ("fp8", "raw"):
            try:
                out, _ = _run_device(x_i, x_j, e_ij, W, b, mode=mode)
                return out
            except Exception as e:  # fail safe: correct > fast
                print(f"kernel: device path '{mode}' failed "
                      f"({type(e).__name__}: {e}); trying next",
                      file=sys.stderr)
    return _numpy_fallback(x_i, x_j, e_ij, adj, e_row, W, b)


# revision 8
# speedup vs baseline: 2.1804x; 1.3423x over previous
"""Trainium2 Bass kernel for nn_Attention_53077205844237 (GNN edge softmax).

Computation (reference):
    q   = x_j + e_ij                          # [E, 128]
    w   = tanh(concat([q, x_i], -1) @ W + b)  # [E, 8]
    out = segment_softmax(w, e_row)           # [E, 8], segments = rows

Problem structure (hardcoded): E = 131072 edges, IN = 128, F = 8,
N = 4096 nodes, and e_row = repeat(arange(4096), 32) -- every segment is a
contiguous, 32-edge block.  Since |tanh| < 1, exp() cannot overflow and the
segment-max subtraction is mathematically a no-op -- only a segment *sum*
is needed.  Edges split contiguously across 8 NeuronCores (16384 = 512
whole segments per core): softmax fully local, no collectives.

Fast path ("fp8"): inputs are quantized host-side to fp8 e3m4 (1.8% RMS
element error; the problem tolerance is 2e-2 relative overall) with error
feedback pairing x_j/e_ij: e_ij is quantized AFTER absorbing x_j's
quantization error, so q = x_j + e_ij carries only a single quantization
error.  Per 2048-edge load: DVE adds q = xj + eij (fp8 -> bf16); the PE
runs 2 matmuls per 512-edge chunk (bf16 stationary x bf16/fp8 moving) into
ONE PSUM bank with the 4 chunks partition-stacked at bases {0,32,64,96}
via matmul tile_position, so ACT (tanh, exp) and DVE (32-wide segment sum,
reciprocal, broadcast mul) each run one instruction per bank across 128
partitions instead of 8.  Output is stored bank-stacked ([128, 512] incl.
24 garbage partitions per 32-group) and unshuffled on host.

Fallback ("raw"): the previous all-f32r 3-matmul pipeline (~105 us).
Final fallback: numpy (correct for arbitrary e_row).
"""

import sys
import types
from contextlib import ExitStack

if "/opt/trn_rl_repo" not in sys.path:
    sys.path.insert(0, "/opt/trn_rl_repo")

import numpy as np

# ---------------------------------------------------------------------------
# Optional NTFF-profile hook (used only when _run(trace=True); harmless else).
# ---------------------------------------------------------------------------
if "antenv.axon_hooks" not in sys.modules:
    _hooks_mod = types.ModuleType("antenv.axon_hooks")
    _hook_box = [None]
    _hooks_mod.set_axon_ntff_profile_hook = lambda h: _hook_box.__setitem__(0, h)
    _hooks_mod.get_axon_ntff_profile_hook = lambda: _hook_box[0]
    sys.modules["antenv.axon_hooks"] = _hooks_mod
    try:
        from trn_agent_boot.trn_boot import _ntff_profile_via_ctypes

        _hooks_mod.set_axon_ntff_profile_hook(
            _ntff_profile_via_ctypes("/opt/axon/libaxon_pjrt.so")
        )
    except Exception:
        pass

# Problem constants (hardcoded per the task contract).
E = 131072
IN = 128
F = 8
N_NODES = 4096
DEG = 32
N_CORES = 8
ES = E // N_CORES          # edges per core = 16384

# fp8 pipeline geometry
LD = 2048                  # edges per load == edges per PSUM bank
NB = ES // LD              # loads (= banks) per core = 8
CH = 512                   # psum chunk (col-tile) width
NCT = LD // CH             # chunks per bank = 4
NIN = 4                    # input ring slots
NQ = 3                     # q ring slots
NEW = 4                    # ew ring slots
NWK = 3                    # out ring slots

SAFE_INTRA = True          # same-engine RAW sem waits (walrus emits DRAINs)

_COMPILED = {}             # mode -> compiled bass module


def _build_fp8():
    import concourse.bacc as bacc
    from concourse import mybir

    f32 = mybir.dt.float32
    bf16 = mybir.dt.bfloat16
    f8 = mybir.dt.float8e3
    AF = mybir.ActivationFunctionType

    nc = bacc.Bacc("TRN2", target_bir_lowering=False, debug=False,
                   num_devices=N_CORES)

    # Packed input: per load b, columns [3*LD*b, 3*LD*(b+1)) hold the three
    # tensors side by side ([xj | eij | xi], LD columns each) so one DMA
    # descriptor with 6 KB contiguous rows covers a whole load.
    pk = nc.dram_tensor("pk", [IN, NB * 3 * LD], f8, kind="ExternalInput")
    w1 = nc.dram_tensor("W1", [IN, F], bf16, kind="ExternalInput")
    w2 = nc.dram_tensor("W2", [IN, F], bf16, kind="ExternalInput")
    bv = nc.dram_tensor("b", [IN, 1], f32, kind="ExternalInput")  # tiled bias
    # Stacked output: out2[32*c + f, CH*b + e] = out[edge LD*b + CH*c + e, f].
    # Partitions 8..31 of each 32-group are garbage (never read by host).
    out2 = nc.dram_tensor("out2", [IN, NB * CH], bf16, kind="ExternalOutput")

    with ExitStack() as ctx:
        def mksem(name):
            return ctx.enter_context(nc.semaphore(name))

        s_in = [mksem(f"s_in{r}") for r in range(NIN)]
        s_out = [mksem(f"s_out{r}") for r in range(NWK)]
        s_mm = mksem("s_mm")
        s_psf = mksem("s_psf")
        s_exp = mksem("s_exp")
        s_red = mksem("s_red")
        s_rcp = mksem("s_rcp")
        s_mul = mksem("s_mul")
        s_const = mksem("s_const")

        in_t = [ctx.enter_context(nc.sbuf_tensor(f"in{r}", [IN, 3 * LD], f8))
                for r in range(NIN)]
        w_t = [ctx.enter_context(nc.sbuf_tensor(f"w{r}", [IN, CH], f32))
               for r in range(2)]
        ew_t = [ctx.enter_context(nc.sbuf_tensor(f"ew{r}", [IN, CH], bf16))
                for r in range(NEW)]
        o_t = [ctx.enter_context(nc.sbuf_tensor(f"o{r}", [IN, CH], bf16))
               for r in range(NWK)]
        dn_t = [ctx.enter_context(nc.sbuf_tensor(f"dn{r}", [IN, CH // DEG],
                                                  bf16)) for r in range(2)]
        rc_t = [ctx.enter_context(nc.sbuf_tensor(f"rc{r}", [IN, CH // DEG],
                                                 bf16)) for r in range(2)]
        ps_t = [ctx.enter_context(nc.psum_tensor(f"ps{r}", [IN, CH], f32))
                for r in range(NB)]
        w1_t = ctx.enter_context(nc.sbuf_tensor("w1s", [IN, F], bf16))
        w2_t = ctx.enter_context(nc.sbuf_tensor("w2s", [IN, F], bf16))
        b_t = ctx.enter_context(nc.sbuf_tensor("bs", [IN, 1], f32))

        def pk_slice(b):
            return slice(b * 3 * LD, (b + 1) * 3 * LD)

        with nc.Block() as block:

            @block.sync
            def _(sp):
                for b in range(0, NB, 2):
                    r = b % NIN
                    if b >= NIN:
                        sp.wait_ge(s_mm, b - NIN + 1)
                    sp.dma_start(out=in_t[r][:], in_=pk[:, pk_slice(b)]
                                 ).then_inc(s_in[r], 16)

            @block.scalar
            def _(act):
                def act_exp(j):
                    # exp of bank j, one bank behind tanh: the gap gives the
                    # producer writes time to land after their sem fires
                    if SAFE_INTRA:
                        act.wait_ge(s_psf, j + 1)
                    if j >= NEW:
                        act.wait_ge(s_mul, j - NEW + 1)
                    act.activation(
                        out=ew_t[j % NEW][:], in_=w_t[j % 2][:],
                        func=AF.Exp,
                    ).then_inc(s_exp, 1)

                def act_bank(bb):
                    act.wait_ge(s_mm, bb + 1)
                    if bb >= 1:
                        act_exp(bb - 1)
                    act.activation(
                        out=w_t[bb % 2][:], in_=ps_t[bb][:],
                        func=AF.Tanh, bias=b_t[:, 0:1],
                    ).then_inc(s_psf, 1)

                LAG = 2
                for b in range(1, NB, 2):
                    r = b % NIN
                    if b >= NIN:
                        act.wait_ge(s_mm, b - NIN + 1)
                    act.dma_start(out=in_t[r][:], in_=pk[:, pk_slice(b)]
                                  ).then_inc(s_in[r], 16)
                    if b == 1:
                        act.wait_ge(s_const, 48)
                    for bb in (b - LAG - 1, b - LAG):
                        if bb >= 0:
                            act_bank(bb)
                for bb in range(NB - LAG, NB):
                    act_bank(bb)
                act_exp(NB - 1)

            @block.tensor
            def _(pe):
                pe.wait_ge(s_const, 32)
                for b in range(NB):
                    r = b % NIN
                    pe.wait_ge(s_in[r], 16 * (b // NIN + 1))
                    ps = ps_t[b]
                    it = in_t[r]
                    for c in range(NCT):
                        po = ps[32 * c:32 * c + F, :]
                        csl = slice(c * CH, (c + 1) * CH)
                        esl = slice(LD + c * CH, LD + (c + 1) * CH)
                        isl = slice(2 * LD + c * CH, 2 * LD + (c + 1) * CH)
                        pe.matmul(po, w1_t[:], it[:, csl],
                                  start=True, stop=False,
                                  tile_position=(0, 32 * c))
                        pe.matmul(po, w1_t[:], it[:, esl],
                                  start=False, stop=False,
                                  tile_position=(0, 32 * c))
                        last = pe.matmul(po, w2_t[:], it[:, isl],
                                         start=False, stop=True,
                                         tile_position=(0, 32 * c))
                    last.then_inc(s_mm, 1)

            @block.vector
            def _(dve):
                nseg = CH // DEG

                def dve_mul(j):
                    # mul of bank j, one bank behind reduce/recip: the gap
                    # gives producer writes time to land after their sem fires
                    if SAFE_INTRA:
                        dve.wait_ge(s_rcp, j + 1)
                    if j >= NWK:
                        dve.wait_ge(s_out[j % NWK],
                                    16 * ((j - NWK) // NWK + 1))
                    dve.tensor_mul(
                        out=o_t[j % NWK][:].rearrange("p (n d) -> p n d",
                                                      d=DEG),
                        in0=ew_t[j % NEW][:].rearrange("p (n d) -> p n d",
                                                       d=DEG),
                        in1=rc_t[j % 2][:].unsqueeze(-1).broadcast_to(
                            [IN, nseg, DEG]),
                    ).then_inc(s_mul, 1)

                for bb in range(NB):
                    dve.wait_ge(s_exp, bb + 1)
                    if bb >= 1:
                        dve_mul(bb - 1)
                    with nc.allow_low_precision(
                            "32-wide segment sum of exp(tanh) in bf16; "
                            "error budget allows ~0.4%"):
                        dve.reduce_sum(
                            out=dn_t[bb % 2][:],
                            in_=ew_t[bb % NEW][:].rearrange(
                                "p (n d) -> p n d", d=DEG),
                            axis=mybir.AxisListType.X,
                        ).then_inc(s_red, 1)
                        if SAFE_INTRA:
                            dve.wait_ge(s_red, bb + 1)
                        dve.reciprocal(out=rc_t[bb % 2][:],
                                       in_=dn_t[bb % 2][:]
                                       ).then_inc(s_rcp, 1)
                dve_mul(NB - 1)

            @block.gpsimd
            def _(gp):
                gp.dma_start(out=w1_t[:], in_=w1[:]).then_inc(s_const, 16)
                gp.dma_start(out=w2_t[:], in_=w2[:]).then_inc(s_const, 16)
                gp.dma_start(out=b_t[:], in_=bv[:]).then_inc(s_const, 16)
                for b in range(NB):
                    gp.wait_ge(s_mul, b + 1)
                    gp.dma_start(out=out2[:, b * CH:(b + 1) * CH],
                                 in_=o_t[b % NWK][:]
                                 ).then_inc(s_out[b % NWK], 16)
                for r in range(NWK):
                    n_r = len(range(r, NB, NWK))
                    gp.wait_ge(s_out[r], 16 * n_r)

    nc.compile()
    return nc


# ---------------------------------------------------------------------------
# Fallback: previous all-f32r raw pipeline (measured ~105 us end-to-end).
# ---------------------------------------------------------------------------
LDR = 2048                 # raw-path input DMA batch (edges)
CHR = 512                  # raw-path matmul chunk


def _load_plan_raw():
    tail = [CHR, CHR, CHR, CHR // 2, CHR // 4, CHR // 4]
    loads = []
    pos = 0
    while pos < ES - sum(tail):
        loads.append((pos, LDR))
        pos += LDR
    for sz in tail:
        loads.append((pos, sz))
        pos += sz
    assert pos == ES, (pos, ES)
    return loads


def _build_bass_raw():
    import concourse.bacc as bacc
    from concourse import mybir

    f32 = mybir.dt.float32
    f32r = mybir.dt.float32r
    AF = mybir.ActivationFunctionType

    nc = bacc.Bacc("TRN2", target_bir_lowering=False, debug=False,
                   num_devices=N_CORES)

    xjT = nc.dram_tensor("xjT", [IN, ES], f32r, kind="ExternalInput")
    eijT = nc.dram_tensor("eijT", [IN, ES], f32r, kind="ExternalInput")
    xiT = nc.dram_tensor("xiT", [IN, ES], f32r, kind="ExternalInput")
    w1 = nc.dram_tensor("W1", [IN, F], f32r, kind="ExternalInput")
    w2 = nc.dram_tensor("W2", [IN, F], f32r, kind="ExternalInput")
    bv = nc.dram_tensor("b", [F, 1], f32, kind="ExternalInput")
    outT = nc.dram_tensor("outT", [F, ES], f32, kind="ExternalOutput")

    loads = _load_plan_raw()
    NB_ = len(loads)
    NIN_ = 5
    NWK_ = 3
    NEW_ = NWK_

    with ExitStack() as ctx:
        def mksem(name):
            return ctx.enter_context(nc.semaphore(name))

        s_xj = [mksem(f"s_xj{r}") for r in range(NIN_)]
        s_eij = [mksem(f"s_eij{r}") for r in range(NIN_)]
        s_xi = [mksem(f"s_xi{r}") for r in range(NIN_)]
        s_out = [mksem(f"s_out{r}") for r in range(NWK_)]
        s_mm = mksem("s_mm")
        s_red = mksem("s_red")
        s_rcp = mksem("s_rcp")
        s_psf = mksem("s_psf")
        s_exp = mksem("s_exp")
        s_mul = mksem("s_mul")
        s_const = mksem("s_const")

        in_xj = [ctx.enter_context(nc.sbuf_tensor(f"in_xj{r}", [IN, LDR], f32r))
                 for r in range(NIN_)]
        in_eij = [ctx.enter_context(nc.sbuf_tensor(f"in_eij{r}", [IN, LDR], f32r))
                  for r in range(NIN_)]
        in_xi = [ctx.enter_context(nc.sbuf_tensor(f"in_xi{r}", [IN, LDR], f32r))
                 for r in range(NIN_)]
        w_t = [ctx.enter_context(nc.sbuf_tensor(f"w{r}", [F, LDR], f32))
               for r in range(NWK_)]
        ew_t = [ctx.enter_context(nc.sbuf_tensor(f"ew{r}", [F, LDR], f32))
                for r in range(NEW_)]
        o_t = [ctx.enter_context(nc.sbuf_tensor(f"o{r}", [F, LDR], f32))
               for r in range(NWK_)]
        dn_t = ctx.enter_context(nc.sbuf_tensor("dn", [F, LDR // DEG], f32))
        rc_t = ctx.enter_context(nc.sbuf_tensor("rc", [F, LDR // DEG], f32))
        ps_t = [ctx.enter_context(nc.psum_tensor(f"ps{r}", [F, LDR], f32))
                for r in range(2)]
        w1_t = ctx.enter_context(nc.sbuf_tensor("w1s", [IN, F], f32r))
        w2_t = ctx.enter_context(nc.sbuf_tensor("w2s", [IN, F], f32r))
        b_t = ctx.enter_context(nc.sbuf_tensor("bs", [F, 1], f32))

        with nc.Block() as block:

            @block.sync
            def _(sp):
                for b, (pos, size) in enumerate(loads):
                    sl = slice(pos, pos + size)
                    if b >= NIN_:
                        sp.wait_ge(s_mm, b - (NIN_ - 1))
                    sp.dma_start(out=in_xj[b % NIN_][:, 0:size],
                                 in_=xjT[:, sl]).then_inc(s_xj[b % NIN_], 16)
                    if b % 2 == 0:
                        sp.dma_start(out=in_xi[b % NIN_][:, 0:size],
                                     in_=xiT[:, sl]).then_inc(s_xi[b % NIN_], 16)

            @block.scalar
            def _(act):
                def act_tail(bb):
                    bsz = loads[bb][1]
                    act.wait_ge(s_mm, bb + 1)
                    if bb >= NEW_:
                        act.wait_ge(s_mul, bb - (NEW_ - 1))
                    act.activation(
                        out=w_t[bb % NWK_][:, 0:bsz],
                        in_=ps_t[bb % 2][:, 0:bsz],
                        func=AF.Tanh, bias=b_t[:, 0:1],
                    ).then_inc(s_psf, 1)
                    if SAFE_INTRA:
                        act.wait_ge(s_psf, bb + 1)
                    act.activation(
                        out=ew_t[bb % NEW_][:, 0:bsz],
                        in_=w_t[bb % NWK_][:, 0:bsz],
                        func=AF.Exp,
                    ).then_inc(s_exp, 1)

                for b, (pos, size) in enumerate(loads):
                    sl = slice(pos, pos + size)
                    if b >= NIN_:
                        act.wait_ge(s_mm, b - (NIN_ - 1))
                    act.dma_start(out=in_eij[b % NIN_][:, 0:size],
                                  in_=eijT[:, sl]).then_inc(s_eij[b % NIN_], 16)
                    if b % 2 == 1:
                        act.dma_start(out=in_xi[b % NIN_][:, 0:size],
                                      in_=xiT[:, sl]).then_inc(s_xi[b % NIN_], 16)
                    if b >= 2:
                        bb = b - 2
                        if bb == 0:
                            act.wait_ge(s_const, 48)
                        act_tail(bb)
                for bb in (NB_ - 2, NB_ - 1):
                    act_tail(bb)

            @block.tensor
            def _(pe):
                pe.wait_ge(s_const, 48)
                for b, (pos, size) in enumerate(loads):
                    r = b % NIN_
                    n_use = b // NIN_ + 1
                    pe.wait_ge(s_xj[r], 16 * n_use)
                    pe.wait_ge(s_eij[r], 16 * n_use)
                    pe.wait_ge(s_xi[r], 16 * n_use)
                    if b >= 2:
                        pe.wait_ge(s_psf, b - 1)
                    ps = ps_t[b % 2]
                    nch = (size + CHR - 1) // CHR
                    for c in range(nch):
                        cw = min(CHR, size - c * CHR)
                        csl = slice(c * CHR, c * CHR + cw)
                        pe.matmul(ps[:, csl],
                                  w1_t[:], in_xj[b % NIN_][:, csl],
                                  start=True, stop=False)
                        pe.matmul(ps[:, csl],
                                  w1_t[:], in_eij[b % NIN_][:, csl],
                                  start=False, stop=False)
                        last = pe.matmul(ps[:, csl],
                                         w2_t[:], in_xi[b % NIN_][:, csl],
                                         start=False, stop=True)
                    last.then_inc(s_mm, 1)

            @block.vector
            def _(dve):
                for b, (pos, size) in enumerate(loads):
                    nseg = size // DEG
                    dve.wait_ge(s_exp, b + 1)
                    ew = ew_t[b % NEW_]
                    dve.reduce_sum(
                        out=dn_t[:, 0:nseg],
                        in_=ew[:, 0:size].rearrange("p (n d) -> p n d", d=DEG),
                        axis=mybir.AxisListType.X,
                    ).then_inc(s_red, 1)
                    if SAFE_INTRA:
                        dve.wait_ge(s_red, b + 1)
                    dve.reciprocal(
                        out=rc_t[:, 0:nseg], in_=dn_t[:, 0:nseg]
                    ).then_inc(s_rcp, 1)
                    if SAFE_INTRA:
                        dve.wait_ge(s_rcp, b + 1)
                    if b >= NWK_:
                        dve.wait_ge(s_out[b % NWK_], 16 * ((b - NWK_) // NWK_ + 1))
                    dve.tensor_mul(
                        out=o_t[b % NWK_][:, 0:size].rearrange(
                            "p (n d) -> p n d", d=DEG),
                        in0=ew[:, 0:size].rearrange("p (n d) -> p n d", d=DEG),
                        in1=rc_t[:, 0:nseg].unsqueeze(-1).broadcast_to(
                            [F, nseg, DEG]),
                    ).then_inc(s_mul, 1)

            @block.gpsimd
            def _(gp):
                gp.dma_start(out=w1_t[:], in_=w1[:]).then_inc(s_const, 16)
                gp.dma_start(out=w2_t[:], in_=w2[:]).then_inc(s_const, 16)
                gp.dma_start(out=b_t[:], in_=bv[:]).then_inc(s_const, 16)
                for b, (pos, size) in enumerate(loads):
                    sl = slice(pos, pos + size)
                    gp.wait_ge(s_mul, b + 1)
                    gp.dma_start(
                        out=outT[:, sl],
                        in_=o_t[b % NWK_][:, 0:size],
                    ).then_inc(s_out[b % NWK_], 16)
                for r in range(NWK_):
                    n_r = len(range(r, NB_, NWK_))
                    gp.wait_ge(s_out[r], 16 * n_r)

    nc.compile()
    return nc


def _get_compiled(mode):
    if mode not in _COMPILED:
        _COMPILED[mode] = _build_fp8() if mode == "fp8" else _build_bass_raw()
    return _COMPILED[mode]


def _prep_inputs_fp8(x_i, x_j, e_ij, W, b):
    import ml_dtypes

    F8 = ml_dtypes.float8_e3m4
    BF16 = ml_dtypes.bfloat16

    W = np.asarray(W, dtype=np.float32)
    W1 = np.ascontiguousarray(W[:IN]).astype(BF16)
    W2 = np.ascontiguousarray(W[IN:]).astype(BF16)
    bias = np.asarray(b, dtype=np.float32).reshape(F)
    btile = np.zeros((IN, 1), np.float32)
    for cc in range(NCT):
        btile[32 * cc:32 * cc + F, 0] = bias

    in_maps = []
    for c in range(N_CORES):
        sl = slice(c * ES, (c + 1) * ES)
        xjT = np.ascontiguousarray(np.asarray(x_j[sl], np.float32).T)
        eijT = np.ascontiguousarray(np.asarray(e_ij[sl], np.float32).T)
        xiT = np.ascontiguousarray(np.asarray(x_i[sl], np.float32).T)
        xj8 = xjT.astype(F8)
        # error feedback: fold xj's quantization error into eij before its
        # quantization, so q = xj + eij carries a single quantization error
        eij8 = (eijT + (xjT - xj8.astype(np.float32))).astype(F8)
        xi8 = xiT.astype(F8)
        # pack per load: [xj | eij | xi] blocks of LD columns each
        pk = np.stack([xj8.reshape(IN, NB, LD),
                       eij8.reshape(IN, NB, LD),
                       xi8.reshape(IN, NB, LD)], axis=2)
        pk = np.ascontiguousarray(pk).reshape(IN, NB * 3 * LD)
        in_maps.append({
            "pk": pk,
            "W1": W1,
            "W2": W2,
            "b": btile,
        })
    return in_maps


def _gather_fp8(res):
    out = np.empty((E, F), dtype=np.float32)
    for c in range(N_CORES):
        o2 = np.asarray(res.results[c]["out2"]).astype(np.float32)
        o4 = o2.reshape(NCT, 32, NB, CH)[:, :F]              # [c, f, b, e]
        out[c * ES:(c + 1) * ES] = o4.transpose(2, 0, 3, 1).reshape(ES, F)
    return out


def _prep_inputs_raw(x_i, x_j, e_ij, W, b):
    W = np.ascontiguousarray(np.asarray(W, dtype=np.float32))
    bias = np.asarray(b, dtype=np.float32).reshape(F, 1)
    W1 = np.ascontiguousarray(W[:IN])
    W2 = np.ascontiguousarray(W[IN:])
    in_maps = []
    for c in range(N_CORES):
        sl = slice(c * ES, (c + 1) * ES)
        in_maps.append({
            "xjT": np.ascontiguousarray(np.asarray(x_j[sl]).T),
            "eijT": np.ascontiguousarray(np.asarray(e_ij[sl]).T),
            "xiT": np.ascontiguousarray(np.asarray(x_i[sl]).T),
            "W1": W1,
            "W2": W2,
            "b": bias,
        })
    return in_maps


def _gather_raw(res):
    out = np.empty((E, F), dtype=np.float32)
    for c in range(N_CORES):
        out[c * ES:(c + 1) * ES] = np.asarray(res.results[c]["outT"]).T
    return out


def _run_device(x_i, x_j, e_ij, W, b, trace=False, tmpdir=None,
                trace_cores=None, mode="fp8"):
    from concourse.bass_utils import run_bass_kernel_spmd

    nc = _get_compiled(mode)
    if mode == "fp8":
        in_maps = _prep_inputs_fp8(x_i, x_j, e_ij, W, b)
    else:
        in_maps = _prep_inputs_raw(x_i, x_j, e_ij, W, b)

    kwargs = {}
    if trace:
        kwargs.update(trace=True,
                      trace_cores=(trace_cores if trace_cores is not None
                                   else list(range(N_CORES))),
                      tmpdir=tmpdir)
    res = run_bass_kernel_spmd(nc, in_maps, core_ids=list(range(N_CORES)),
                               **kwargs)

    out = _gather_fp8(res) if mode == "fp8" else _gather_raw(res)
    return out, res


def _numpy_fallback(x_i, x_j, e_ij, adj, e_row, W, b):
    """Correct for arbitrary e_row (matches the reference semantics)."""
    x_i = np.asarray(x_i, np.float32)
    x_j = np.asarray(x_j, np.float32)
    e_ij = np.asarray(e_ij, np.float32)
    W = np.asarray(W, np.float32)
    b = np.asarray(b, np.float32)
    e_row = np.asarray(e_row).astype(np.int64)
    n = np.asarray(adj).shape[0]
    q = x_j + e_ij
    z = q @ W[:q.shape[1]] + x_i @ W[q.shape[1]:] + b
    w = np.tanh(z)
    m = np.full((n, w.shape[1]), -9e15, np.float32)
    np.maximum.at(m, e_row, w)
    ew = np.exp(w - m[e_row])
    denom = np.zeros((n, w.shape[1]), np.float32)
    np.add.at(denom, e_row, ew)
    return (ew / denom[e_row]).astype(np.float32)


def _is_fast_path(x_i, x_j, e_ij, adj, e_row, W, b):
    try:
        if np.asarray(x_i).shape != (E, IN):
            return False
        if np.asarray(x_j).shape != (E, IN):
            return False
        if np.asarray(e_ij).shape != (E, IN):
            return False
        if np.asarray(W).shape != (2 * IN, F):
            return False
        if np.asarray(b).reshape(-1).shape != (F,):
            return False
        if np.asarray(adj).shape[0] != N_NODES:
            return False
        er = np.asarray(e_row).reshape(-1)
        if er.shape != (E,):
            return False
        expected = np.repeat(np.arange(N_NODES, dtype=np.int64), DEG)
        return bool(np.array_equal(er.astype(np.int64), expected))
    except Exception:
        return False


def kernel(x_i, x_j, e_ij, adj, e_row, e_col, W, b, **_unused):
    if _is_fast_path(x_i, x_j, e_ij, adj, e_row, W, b):
        for mode in ("fp8", "raw"):
            try:
                out, _ = _run_device(x_i, x_j, e_ij, W, b, mode=mode)
                return out
            except Exception as e:  # fail safe: correct > fast
                print(f"kernel: device path '{mode}' failed "
                      f"({type(e).__name__}: {e}); trying next",
                      file=sys.stderr)
    return _numpy_fallback(x_i, x_j, e_ij, adj, e_row, W, b)


# revision 9
# speedup vs baseline: 2.2690x; 1.0406x over previous
"""Trainium2 Bass kernel for nn_Attention_53077205844237 (GNN edge softmax).

Computation (reference):
    q   = x_j + e_ij                          # [E, 128]
    w   = tanh(concat([q, x_i], -1) @ W + b)  # [E, 8]
    out = segment_softmax(w, e_row)           # [E, 8], segments = rows

Problem structure (hardcoded): E = 131072 edges, IN = 128, F = 8,
N = 4096 nodes, and e_row = repeat(arange(4096), 32) -- every segment is a
contiguous, 32-edge block.  Since |tanh| < 1, exp() cannot overflow and the
segment-max subtraction is mathematically a no-op -- only a segment *sum*
is needed.  Edges split contiguously across 8 NeuronCores (16384 = 512
whole segments per core): softmax fully local, no collectives.

Fast path ("fp8"): inputs are quantized host-side to fp8 e3m4 (1.8% RMS
element error; the problem tolerance is 2e-2 relative overall) with error
feedback pairing x_j/e_ij: e_ij is quantized AFTER absorbing x_j's
quantization error, so q = x_j + e_ij carries only a single quantization
error.  Per 2048-edge load: DVE adds q = xj + eij (fp8 -> bf16); the PE
runs 2 matmuls per 512-edge chunk (bf16 stationary x bf16/fp8 moving) into
ONE PSUM bank with the 4 chunks partition-stacked at bases {0,32,64,96}
via matmul tile_position, so ACT (tanh, exp) and DVE (32-wide segment sum,
reciprocal, broadcast mul) each run one instruction per bank across 128
partitions instead of 8.  Output is stored bank-stacked ([128, 512] incl.
24 garbage partitions per 32-group) and unshuffled on host.

Fallback ("raw"): the previous all-f32r 3-matmul pipeline (~105 us).
Final fallback: numpy (correct for arbitrary e_row).
"""

import sys
import types
from contextlib import ExitStack

if "/opt/trn_rl_repo" not in sys.path:
    sys.path.insert(0, "/opt/trn_rl_repo")

import numpy as np

# ---------------------------------------------------------------------------
# Optional NTFF-profile hook (used only when _run(trace=True); harmless else).
# ---------------------------------------------------------------------------
if "antenv.axon_hooks" not in sys.modules:
    _hooks_mod = types.ModuleType("antenv.axon_hooks")
    _hook_box = [None]
    _hooks_mod.set_axon_ntff_profile_hook = lambda h: _hook_box.__setitem__(0, h)
    _hooks_mod.get_axon_ntff_profile_hook = lambda: _hook_box[0]
    sys.modules["antenv.axon_hooks"] = _hooks_mod
    try:
        from trn_agent_boot.trn_boot import _ntff_profile_via_ctypes

        _hooks_mod.set_axon_ntff_profile_hook(
            _ntff_profile_via_ctypes("/opt/axon/libaxon_pjrt.so")
        )
    except Exception:
        pass

# Problem constants (hardcoded per the task contract).
E = 131072
IN = 128
F = 8
N_NODES = 4096
DEG = 32
N_CORES = 8
ES = E // N_CORES          # edges per core = 16384

# fp8 pipeline geometry
LD = 2048                  # edges per load == edges per PSUM bank
NB = ES // LD              # loads (= banks) per core = 8
CH = 512                   # psum chunk (col-tile) width
NCT = LD // CH             # chunks per bank = 4
NIN = 4                    # input ring slots
NQ = 3                     # q ring slots
NEW = 4                    # ew ring slots
NWK = 3                    # out ring slots

SAFE_INTRA = True          # same-engine RAW sem waits (walrus emits DRAINs)

_COMPILED = {}             # mode -> compiled bass module


def _build_fp8():
    import concourse.bacc as bacc
    from concourse import mybir

    f32 = mybir.dt.float32
    bf16 = mybir.dt.bfloat16
    f8 = mybir.dt.float8e3
    AF = mybir.ActivationFunctionType

    nc = bacc.Bacc("TRN2", target_bir_lowering=False, debug=False,
                   num_devices=N_CORES)

    # Packed input: load b occupies rows [IN*b, IN*(b+1)) -- a fully
    # contiguous 768 KB DRAM block per load ([xj | eij | xi], LD cols each)
    # so the DMA engines see one linear read per load.
    pk = nc.dram_tensor("pk", [NB * IN, 3 * LD], f8, kind="ExternalInput")
    w1 = nc.dram_tensor("W1", [IN, F], bf16, kind="ExternalInput")
    w2 = nc.dram_tensor("W2", [IN, F], bf16, kind="ExternalInput")
    bv = nc.dram_tensor("b", [IN, 1], f32, kind="ExternalInput")  # tiled bias
    # Stacked output: out2[32*c + f, CH*b + e] = out[edge LD*b + CH*c + e, f].
    # Partitions 8..31 of each 32-group are garbage (never read by host).
    out2 = nc.dram_tensor("out2", [IN, NB * CH], bf16, kind="ExternalOutput")

    # Pipeline-counter semaphores (one per producer engine, computed
    # thresholds; DMA sems are per-queue, sound because each queue
    # completes its transfers in issue order):
    #   s_act: +1 per ACT activation   tanh[bb] -> 2bb+1, exp[j] -> 2j+2
    #   s_dve: +1 per DVE op           red[bb] -> 3bb+1, rcp[bb] -> 3bb+2,
    #                                  mul[j] -> 3j+3
    with ExitStack() as ctx:
        def mksem(name):
            return ctx.enter_context(nc.semaphore(name))

        s_in = [mksem("s_in_sp"), mksem("s_in_act")]
        s_out = mksem("s_out")
        s_mm = mksem("s_mm")
        s_act = mksem("s_act")
        s_dve = mksem("s_dve")
        s_const = mksem("s_const")

        in_t = [ctx.enter_context(nc.sbuf_tensor(f"in{r}", [IN, 3 * LD], f8))
                for r in range(NIN)]
        w_t = [ctx.enter_context(nc.sbuf_tensor(f"w{r}", [IN, CH], f32))
               for r in range(2)]
        ew_t = [ctx.enter_context(nc.sbuf_tensor(f"ew{r}", [IN, CH], bf16))
                for r in range(NEW)]
        o_t = [ctx.enter_context(nc.sbuf_tensor(f"o{r}", [IN, CH], bf16))
               for r in range(NWK)]
        dn_t = [ctx.enter_context(nc.sbuf_tensor(f"dn{r}", [IN, CH // DEG],
                                                 bf16)) for r in range(2)]
        rc_t = [ctx.enter_context(nc.sbuf_tensor(f"rc{r}", [IN, CH // DEG],
                                                 bf16)) for r in range(2)]
        ps_t = [ctx.enter_context(nc.psum_tensor(f"ps{r}", [IN, CH], f32))
                for r in range(NB)]
        w1_t = ctx.enter_context(nc.sbuf_tensor("w1s", [IN, F], bf16))
        w2_t = ctx.enter_context(nc.sbuf_tensor("w2s", [IN, F], bf16))
        b_t = ctx.enter_context(nc.sbuf_tensor("bs", [IN, 1], f32))

        def pk_rows(b):
            return pk[b * IN:(b + 1) * IN, :]

        with nc.Block() as block:

            @block.sync
            def _(sp):
                for b in range(0, NB, 2):
                    r = b % NIN
                    if b >= NIN:
                        sp.wait_ge(s_mm, b - NIN + 1)
                    sp.dma_start(out=in_t[r][:], in_=pk_rows(b)
                                 ).then_inc(s_in[0], 16)

            @block.scalar
            def _(act):
                def act_exp(j):
                    # exp of bank j, one bank behind tanh: the gap gives the
                    # producer writes time to land after their sem fires
                    if SAFE_INTRA:
                        act.wait_ge(s_act, 2 * j + 1)
                    if j >= NEW:
                        act.wait_ge(s_dve, 3 * (j - NEW) + 3)
                    act.activation(
                        out=ew_t[j % NEW][:], in_=w_t[j % 2][:],
                        func=AF.Exp,
                    ).then_inc(s_act, 1)

                def act_bank(bb):
                    act.wait_ge(s_mm, bb + 1)
                    if bb >= 1:
                        act_exp(bb - 1)
                    act.activation(
                        out=w_t[bb % 2][:], in_=ps_t[bb][:],
                        func=AF.Tanh, bias=b_t[:, 0:1],
                    ).then_inc(s_act, 1)

                LAG = 2
                for b in range(1, NB, 2):
                    r = b % NIN
                    if b >= NIN:
                        act.wait_ge(s_mm, b - NIN + 1)
                    act.dma_start(out=in_t[r][:], in_=pk_rows(b)
                                  ).then_inc(s_in[1], 16)
                    if b == 1:
                        act.wait_ge(s_const, 48)
                    for bb in (b - LAG - 1, b - LAG):
                        if bb >= 0:
                            act_bank(bb)
                for bb in range(NB - LAG, NB):
                    act_bank(bb)
                act_exp(NB - 1)

            @block.tensor
            def _(pe):
                pe.wait_ge(s_const, 32)
                for b in range(NB):
                    r = b % NIN
                    pe.wait_ge(s_in[b % 2], 16 * (b // 2 + 1))
                    ps = ps_t[b]
                    it = in_t[r]
                    for c in range(NCT):
                        po = ps[32 * c:32 * c + F, :]
                        csl = slice(c * CH, (c + 1) * CH)
                        esl = slice(LD + c * CH, LD + (c + 1) * CH)
                        isl = slice(2 * LD + c * CH, 2 * LD + (c + 1) * CH)
                        pe.matmul(po, w1_t[:], it[:, csl],
                                  start=True, stop=False,
                                  tile_position=(0, 32 * c))
                        pe.matmul(po, w1_t[:], it[:, esl],
                                  start=False, stop=False,
                                  tile_position=(0, 32 * c))
                        last = pe.matmul(po, w2_t[:], it[:, isl],
                                         start=False, stop=True,
                                         tile_position=(0, 32 * c))
                    last.then_inc(s_mm, 1)

            @block.vector
            def _(dve):
                nseg = CH // DEG

                def dve_mul(j):
                    # mul of bank j, one bank behind reduce/recip: the gap
                    # gives producer writes time to land after their sem fires
                    if SAFE_INTRA:
                        dve.wait_ge(s_dve, 3 * j + 2)
                    if j >= NWK:
                        dve.wait_ge(s_out, 16 * (j - NWK + 1))
                    dve.tensor_mul(
                        out=o_t[j % NWK][:].rearrange("p (n d) -> p n d",
                                                      d=DEG),
                        in0=ew_t[j % NEW][:].rearrange("p (n d) -> p n d",
                                                       d=DEG),
                        in1=rc_t[j % 2][:].unsqueeze(-1).broadcast_to(
                            [IN, nseg, DEG]),
                    ).then_inc(s_dve, 1)

                for bb in range(NB):
                    dve.wait_ge(s_act, 2 * bb + 2)
                    if bb >= 1:
                        dve_mul(bb - 1)
                    with nc.allow_low_precision(
                            "32-wide segment sum of exp(tanh) in bf16; "
                            "error budget allows ~0.4%"):
                        dve.reduce_sum(
                            out=dn_t[bb % 2][:],
                            in_=ew_t[bb % NEW][:].rearrange(
                                "p (n d) -> p n d", d=DEG),
                            axis=mybir.AxisListType.X,
                        ).then_inc(s_dve, 1)
                        if SAFE_INTRA:
                            dve.wait_ge(s_dve, 3 * bb + 1)
                        dve.reciprocal(out=rc_t[bb % 2][:],
                                       in_=dn_t[bb % 2][:]
                                       ).then_inc(s_dve, 1)
                dve_mul(NB - 1)

            @block.gpsimd
            def _(gp):
                gp.dma_start(out=w1_t[:], in_=w1[:]).then_inc(s_const, 16)
                gp.dma_start(out=w2_t[:], in_=w2[:]).then_inc(s_const, 16)
                gp.dma_start(out=b_t[:], in_=bv[:]).then_inc(s_const, 16)
                for b in range(NB):
                    gp.wait_ge(s_dve, 3 * b + 3)
                    gp.dma_start(out=out2[:, b * CH:(b + 1) * CH],
                                 in_=o_t[b % NWK][:]
                                 ).then_inc(s_out, 16)
                gp.wait_ge(s_out, 16 * NB)

    nc.compile()
    return nc


# ---------------------------------------------------------------------------
# Fallback: previous all-f32r raw pipeline (measured ~105 us end-to-end).
# ---------------------------------------------------------------------------
LDR = 2048                 # raw-path input DMA batch (edges)
CHR = 512                  # raw-path matmul chunk


def _load_plan_raw():
    tail = [CHR, CHR, CHR, CHR // 2, CHR // 4, CHR // 4]
    loads = []
    pos = 0
    while pos < ES - sum(tail):
        loads.append((pos, LDR))
        pos += LDR
    for sz in tail:
        loads.append((pos, sz))
        pos += sz
    assert pos == ES, (pos, ES)
    return loads


def _build_bass_raw():
    import concourse.bacc as bacc
    from concourse import mybir

    f32 = mybir.dt.float32
    f32r = mybir.dt.float32r
    AF = mybir.ActivationFunctionType

    nc = bacc.Bacc("TRN2", target_bir_lowering=False, debug=False,
                   num_devices=N_CORES)

    xjT = nc.dram_tensor("xjT", [IN, ES], f32r, kind="ExternalInput")
    eijT = nc.dram_tensor("eijT", [IN, ES], f32r, kind="ExternalInput")
    xiT = nc.dram_tensor("xiT", [IN, ES], f32r, kind="ExternalInput")
    w1 = nc.dram_tensor("W1", [IN, F], f32r, kind="ExternalInput")
    w2 = nc.dram_tensor("W2", [IN, F], f32r, kind="ExternalInput")
    bv = nc.dram_tensor("b", [F, 1], f32, kind="ExternalInput")
    outT = nc.dram_tensor("outT", [F, ES], f32, kind="ExternalOutput")

    loads = _load_plan_raw()
    NB_ = len(loads)
    NIN_ = 5
    NWK_ = 3
    NEW_ = NWK_

    with ExitStack() as ctx:
        def mksem(name):
            return ctx.enter_context(nc.semaphore(name))

        s_xj = [mksem(f"s_xj{r}") for r in range(NIN_)]
        s_eij = [mksem(f"s_eij{r}") for r in range(NIN_)]
        s_xi = [mksem(f"s_xi{r}") for r in range(NIN_)]
        s_out = [mksem(f"s_out{r}") for r in range(NWK_)]
        s_mm = mksem("s_mm")
        s_red = mksem("s_red")
        s_rcp = mksem("s_rcp")
        s_psf = mksem("s_psf")
        s_exp = mksem("s_exp")
        s_mul = mksem("s_mul")
        s_const = mksem("s_const")

        in_xj = [ctx.enter_context(nc.sbuf_tensor(f"in_xj{r}", [IN, LDR], f32r))
                 for r in range(NIN_)]
        in_eij = [ctx.enter_context(nc.sbuf_tensor(f"in_eij{r}", [IN, LDR], f32r))
                  for r in range(NIN_)]
        in_xi = [ctx.enter_context(nc.sbuf_tensor(f"in_xi{r}", [IN, LDR], f32r))
                 for r in range(NIN_)]
        w_t = [ctx.enter_context(nc.sbuf_tensor(f"w{r}", [F, LDR], f32))
               for r in range(NWK_)]
        ew_t = [ctx.enter_context(nc.sbuf_tensor(f"ew{r}", [F, LDR], f32))
                for r in range(NEW_)]
        o_t = [ctx.enter_context(nc.sbuf_tensor(f"o{r}", [F, LDR], f32))
               for r in range(NWK_)]
        dn_t = ctx.enter_context(nc.sbuf_tensor("dn", [F, LDR // DEG], f32))
        rc_t = ctx.enter_context(nc.sbuf_tensor("rc", [F, LDR // DEG], f32))
        ps_t = [ctx.enter_context(nc.psum_tensor(f"ps{r}", [F, LDR], f32))
                for r in range(2)]
        w1_t = ctx.enter_context(nc.sbuf_tensor("w1s", [IN, F], f32r))
        w2_t = ctx.enter_context(nc.sbuf_tensor("w2s", [IN, F], f32r))
        b_t = ctx.enter_context(nc.sbuf_tensor("bs", [F, 1], f32))

        with nc.Block() as block:

            @block.sync
            def _(sp):
                for b, (pos, size) in enumerate(loads):
                    sl = slice(pos, pos + size)
                    if b >= NIN_:
                        sp.wait_ge(s_mm, b - (NIN_ - 1))
                    sp.dma_start(out=in_xj[b % NIN_][:, 0:size],
                                 in_=xjT[:, sl]).then_inc(s_xj[b % NIN_], 16)
                    if b % 2 == 0:
                        sp.dma_start(out=in_xi[b % NIN_][:, 0:size],
                                     in_=xiT[:, sl]).then_inc(s_xi[b % NIN_], 16)

            @block.scalar
            def _(act):
                def act_tail(bb):
                    bsz = loads[bb][1]
                    act.wait_ge(s_mm, bb + 1)
                    if bb >= NEW_:
                        act.wait_ge(s_mul, bb - (NEW_ - 1))
                    act.activation(
                        out=w_t[bb % NWK_][:, 0:bsz],
                        in_=ps_t[bb % 2][:, 0:bsz],
                        func=AF.Tanh, bias=b_t[:, 0:1],
                    ).then_inc(s_psf, 1)
                    if SAFE_INTRA:
                        act.wait_ge(s_psf, bb + 1)
                    act.activation(
                        out=ew_t[bb % NEW_][:, 0:bsz],
                        in_=w_t[bb % NWK_][:, 0:bsz],
                        func=AF.Exp,
                    ).then_inc(s_exp, 1)

                for b, (pos, size) in enumerate(loads):
                    sl = slice(pos, pos + size)
                    if b >= NIN_:
                        act.wait_ge(s_mm, b - (NIN_ - 1))
                    act.dma_start(out=in_eij[b % NIN_][:, 0:size],
                                  in_=eijT[:, sl]).then_inc(s_eij[b % NIN_], 16)
                    if b % 2 == 1:
                        act.dma_start(out=in_xi[b % NIN_][:, 0:size],
                                      in_=xiT[:, sl]).then_inc(s_xi[b % NIN_], 16)
                    if b >= 2:
                        bb = b - 2
                        if bb == 0:
                            act.wait_ge(s_const, 48)
                        act_tail(bb)
                for bb in (NB_ - 2, NB_ - 1):
                    act_tail(bb)

            @block.tensor
            def _(pe):
                pe.wait_ge(s_const, 48)
                for b, (pos, size) in enumerate(loads):
                    r = b % NIN_
                    n_use = b // NIN_ + 1
                    pe.wait_ge(s_xj[r], 16 * n_use)
                    pe.wait_ge(s_eij[r], 16 * n_use)
                    pe.wait_ge(s_xi[r], 16 * n_use)
                    if b >= 2:
                        pe.wait_ge(s_psf, b - 1)
                    ps = ps_t[b % 2]
                    nch = (size + CHR - 1) // CHR
                    for c in range(nch):
                        cw = min(CHR, size - c * CHR)
                        csl = slice(c * CHR, c * CHR + cw)
                        pe.matmul(ps[:, csl],
                                  w1_t[:], in_xj[b % NIN_][:, csl],
                                  start=True, stop=False)
                        pe.matmul(ps[:, csl],
                                  w1_t[:], in_eij[b % NIN_][:, csl],
                                  start=False, stop=False)
                        last = pe.matmul(ps[:, csl],
                                         w2_t[:], in_xi[b % NIN_][:, csl],
                                         start=False, stop=True)
                    last.then_inc(s_mm, 1)

            @block.vector
            def _(dve):
                for b, (pos, size) in enumerate(loads):
                    nseg = size // DEG
                    dve.wait_ge(s_exp, b + 1)
                    ew = ew_t[b % NEW_]
                    dve.reduce_sum(
                        out=dn_t[:, 0:nseg],
                        in_=ew[:, 0:size].rearrange("p (n d) -> p n d", d=DEG),
                        axis=mybir.AxisListType.X,
                    ).then_inc(s_red, 1)
                    if SAFE_INTRA:
                        dve.wait_ge(s_red, b + 1)
                    dve.reciprocal(
                        out=rc_t[:, 0:nseg], in_=dn_t[:, 0:nseg]
                    ).then_inc(s_rcp, 1)
                    if SAFE_INTRA:
                        dve.wait_ge(s_rcp, b + 1)
                    if b >= NWK_:
                        dve.wait_ge(s_out[b % NWK_], 16 * ((b - NWK_) // NWK_ + 1))
                    dve.tensor_mul(
                        out=o_t[b % NWK_][:, 0:size].rearrange(
                            "p (n d) -> p n d", d=DEG),
                        in0=ew[:, 0:size].rearrange("p (n d) -> p n d", d=DEG),
                        in1=rc_t[:, 0:nseg].unsqueeze(-1).broadcast_to(
                            [F, nseg, DEG]),
                    ).then_inc(s_mul, 1)

            @block.gpsimd
            def _(gp):
                gp.dma_start(out=w1_t[:], in_=w1[:]).then_inc(s_const, 16)
                gp.dma_start(out=w2_t[:], in_=w2[:]).then_inc(s_const, 16)
                gp.dma_start(out=b_t[:], in_=bv[:]).then_inc(s_const, 16)
                for b, (pos, size) in enumerate(loads):
                    sl = slice(pos, pos + size)
                    gp.wait_ge(s_mul, b + 1)
                    gp.dma_start(
                        out=outT[:, sl],
                        in_=o_t[b % NWK_][:, 0:size],
                    ).then_inc(s_out[b % NWK_], 16)
                for r in range(NWK_):
                    n_r = len(range(r, NB_, NWK_))
                    gp.wait_ge(s_out[r], 16 * n_r)

    nc.compile()
    return nc


def _get_compiled(mode):
    if mode not in _COMPILED:
        _COMPILED[mode] = _build_fp8() if mode == "fp8" else _build_bass_raw()
    return _COMPILED[mode]


def _prep_inputs_fp8(x_i, x_j, e_ij, W, b):
    import ml_dtypes

    F8 = ml_dtypes.float8_e3m4
    BF16 = ml_dtypes.bfloat16

    W = np.asarray(W, dtype=np.float32)
    W1 = np.ascontiguousarray(W[:IN]).astype(BF16)
    W2 = np.ascontiguousarray(W[IN:]).astype(BF16)
    bias = np.asarray(b, dtype=np.float32).reshape(F)
    btile = np.zeros((IN, 1), np.float32)
    for cc in range(NCT):
        btile[32 * cc:32 * cc + F, 0] = bias

    in_maps = []
    for c in range(N_CORES):
        sl = slice(c * ES, (c + 1) * ES)
        xjT = np.ascontiguousarray(np.asarray(x_j[sl], np.float32).T)
        eijT = np.ascontiguousarray(np.asarray(e_ij[sl], np.float32).T)
        xiT = np.ascontiguousarray(np.asarray(x_i[sl], np.float32).T)
        xj8 = xjT.astype(F8)
        # error feedback: fold xj's quantization error into eij before its
        # quantization, so q = xj + eij carries a single quantization error
        eij8 = (eijT + (xjT - xj8.astype(np.float32))).astype(F8)
        xi8 = xiT.astype(F8)
        # pack per load: contiguous [128, 3*LD] block b = [xj | eij | xi]
        pk = np.stack([xj8.reshape(IN, NB, LD),
                       eij8.reshape(IN, NB, LD),
                       xi8.reshape(IN, NB, LD)], axis=2)   # [IN, NB, 3, LD]
        pk = np.ascontiguousarray(pk.transpose(1, 0, 2, 3)
                                  ).reshape(NB * IN, 3 * LD)
        in_maps.append({
            "pk": pk,
            "W1": W1,
            "W2": W2,
            "b": btile,
        })
    return in_maps


def _gather_fp8(res):
    out = np.empty((E, F), dtype=np.float32)
    for c in range(N_CORES):
        o2 = np.asarray(res.results[c]["out2"]).astype(np.float32)
        o4 = o2.reshape(NCT, 32, NB, CH)[:, :F]              # [c, f, b, e]
        out[c * ES:(c + 1) * ES] = o4.transpose(2, 0, 3, 1).reshape(ES, F)
    return out


def _prep_inputs_raw(x_i, x_j, e_ij, W, b):
    W = np.ascontiguousarray(np.asarray(W, dtype=np.float32))
    bias = np.asarray(b, dtype=np.float32).reshape(F, 1)
    W1 = np.ascontiguousarray(W[:IN])
    W2 = np.ascontiguousarray(W[IN:])
    in_maps = []
    for c in range(N_CORES):
        sl = slice(c * ES, (c + 1) * ES)
        in_maps.append({
            "xjT": np.ascontiguousarray(np.asarray(x_j[sl]).T),
            "eijT": np.ascontiguousarray(np.asarray(e_ij[sl]).T),
            "xiT": np.ascontiguousarray(np.asarray(x_i[sl]).T),
            "W1": W1,
            "W2": W2,
            "b": bias,
        })
    return in_maps


def _gather_raw(res):
    out = np.empty((E, F), dtype=np.float32)
    for c in range(N_CORES):
        out[c * ES:(c + 1) * ES] = np.asarray(res.results[c]["outT"]).T
    return out


def _run_device(x_i, x_j, e_ij, W, b, trace=False, tmpdir=None,
                trace_cores=None, mode="fp8"):
    from concourse.bass_utils import run_bass_kernel_spmd

    nc = _get_compiled(mode)
    if mode == "fp8":
        in_maps = _prep_inputs_fp8(x_i, x_j, e_ij, W, b)
    else:
        in_maps = _prep_inputs_raw(x_i, x_j, e_ij, W, b)

    kwargs = {}
    if trace:
        kwargs.update(trace=True,
                      trace_cores=(trace_cores if trace_cores is not None
                                   else list(range(N_CORES))),
                      tmpdir=tmpdir)
    res = run_bass_kernel_spmd(nc, in_maps, core_ids=list(range(N_CORES)),
                               **kwargs)

    out = _gather_fp8(res) if mode == "fp8" else _gather_raw(res)
    return out, res


def _numpy_fallback(x_i, x_j, e_ij, adj, e_row, W, b):
    """Correct for arbitrary e_row (matches the reference semantics)."""
    x_i = np.asarray(x_i, np.float32)
    x_j = np.asarray(x_j, np.float32)
    e_ij = np.asarray(e_ij, np.float32)
    W = np.asarray(W, np.float32)
    b = np.asarray(b, np.float32)
    e_row = np.asarray(e_row).astype(np.int64)
    n = np.asarray(adj).shape[0]
    q = x_j + e_ij
    z = q @ W[:q.shape[1]] + x_i @ W[q.shape[1]:] + b
    w = np.tanh(z)
    m = np.full((n, w.shape[1]), -9e15, np.float32)
    np.maximum.at(m, e_row, w)
    ew = np.exp(w - m[e_row])
    denom = np.zeros((n, w.shape[1]), np.float32)
    np.add.at(denom, e_row, ew)
    return (ew / denom[e_row]).astype(np.float32)


def _is_fast_path(x_i, x_j, e_ij, adj, e_row, W, b):
    try:
        if np.asarray(x_i).shape != (E, IN):
            return False
        if np.asarray(x_j).shape != (E, IN):
            return False
        if np.asarray(e_ij).shape != (E, IN):
            return False
        if np.asarray(W).shape != (2 * IN, F):
            return False
        if np.asarray(b).reshape(-1).shape != (F,):
            return False
        if np.asarray(adj).shape[0] != N_NODES:
            return False
        er = np.asarray(e_row).reshape(-1)
        if er.shape != (E,):
            return False
        expected = np.repeat(np.arange(N_NODES, dtype=np.int64), DEG)
        return bool(np.array_equal(er.astype(np.int64), expected))
    except Exception:
        return False


def kernel(x_i, x_j, e_ij, adj, e_row, e_col, W, b, **_unused):
    if _is_fast_path(x_i, x_j, e_ij, adj, e_row, W, b):
        for mode in ("fp8", "raw"):
            try:
                out, _ = _run_device(x_i, x_j, e_ij, W, b, mode=mode)
                return out
            except Exception as e:  # fail safe: correct > fast
                print(f"kernel: device path '{mode}' failed "
                      f"({type(e).__name__}: {e}); trying next",
                      file=sys.stderr)
    return _numpy_fallback(x_i, x_j, e_ij, adj, e_row, W, b)


# revision 10
# speedup vs baseline: 2.3682x; 1.0437x over previous
"""Trainium2 Bass kernel for nn_Attention_53077205844237 (GNN edge softmax).

Computation (reference):
    q   = x_j + e_ij                          # [E, 128]
    w   = tanh(concat([q, x_i], -1) @ W + b)  # [E, 8]
    out = segment_softmax(w, e_row)           # [E, 8], segments = rows

Problem structure (hardcoded): E = 131072 edges, IN = 128, F = 8,
N = 4096 nodes, and e_row = repeat(arange(4096), 32) -- every segment is a
contiguous, 32-edge block.  Since |tanh| < 1, exp() cannot overflow and the
segment-max subtraction is mathematically a no-op -- only a segment *sum*
is needed.  Edges split contiguously across 8 NeuronCores (16384 = 512
whole segments per core): softmax fully local, no collectives.

Fast path ("fp8"): inputs are quantized host-side to fp8 e3m4 (1.8% RMS
element error; the problem tolerance is 2e-2 relative overall) with error
feedback pairing x_j/e_ij: e_ij is quantized AFTER absorbing x_j's
quantization error, so q = x_j + e_ij carries only a single quantization
error.  Per 2048-edge load: DVE adds q = xj + eij (fp8 -> bf16); the PE
runs 2 matmuls per 512-edge chunk (bf16 stationary x bf16/fp8 moving) into
ONE PSUM bank with the 4 chunks partition-stacked at bases {0,32,64,96}
via matmul tile_position, so ACT (tanh, exp) and DVE (32-wide segment sum,
reciprocal, broadcast mul) each run one instruction per bank across 128
partitions instead of 8.  Output is stored bank-stacked ([128, 512] incl.
24 garbage partitions per 32-group) and unshuffled on host.

Fallback ("raw"): the previous all-f32r 3-matmul pipeline (~105 us).
Final fallback: numpy (correct for arbitrary e_row).
"""

import sys
import types
from contextlib import ExitStack

if "/opt/trn_rl_repo" not in sys.path:
    sys.path.insert(0, "/opt/trn_rl_repo")

import numpy as np

# ---------------------------------------------------------------------------
# Optional NTFF-profile hook (used only when _run(trace=True); harmless else).
# ---------------------------------------------------------------------------
if "antenv.axon_hooks" not in sys.modules:
    _hooks_mod = types.ModuleType("antenv.axon_hooks")
    _hook_box = [None]
    _hooks_mod.set_axon_ntff_profile_hook = lambda h: _hook_box.__setitem__(0, h)
    _hooks_mod.get_axon_ntff_profile_hook = lambda: _hook_box[0]
    sys.modules["antenv.axon_hooks"] = _hooks_mod
    try:
        from trn_agent_boot.trn_boot import _ntff_profile_via_ctypes

        _hooks_mod.set_axon_ntff_profile_hook(
            _ntff_profile_via_ctypes("/opt/axon/libaxon_pjrt.so")
        )
    except Exception:
        pass

# Problem constants (hardcoded per the task contract).
E = 131072
IN = 128
F = 8
N_NODES = 4096
DEG = 32
N_CORES = 8
ES = E // N_CORES          # edges per core = 16384

# fp8 pipeline geometry
LD = 2048                  # edges per load == edges per PSUM bank
NB = ES // LD              # loads (= banks) per core = 8
CH = 512                   # psum chunk (col-tile) width
NCT = LD // CH             # chunks per bank = 4
NIN = 4                    # input ring slots
NQ = 3                     # q ring slots
NEW = 4                    # ew ring slots
NWK = 3                    # out ring slots

SAFE_INTRA = True          # same-engine RAW sem waits (walrus emits DRAINs)

_COMPILED = {}             # mode -> compiled bass module


def _build_fp8():
    import concourse.bacc as bacc
    from concourse import mybir

    f32 = mybir.dt.float32
    bf16 = mybir.dt.bfloat16
    f8 = mybir.dt.float8e3
    AF = mybir.ActivationFunctionType

    nc = bacc.Bacc("TRN2", target_bir_lowering=False, debug=False,
                   num_devices=N_CORES)

    # Packed input: load b occupies rows [IN*b, IN*(b+1)) -- a fully
    # contiguous 768 KB DRAM block per load ([xj | eij | xi], LD cols each)
    # so the DMA engines see one linear read per load.
    pk = nc.dram_tensor("pk", [NB * IN, 3 * LD], f8, kind="ExternalInput")
    w1 = nc.dram_tensor("W1", [IN, F], bf16, kind="ExternalInput")
    w2 = nc.dram_tensor("W2", [IN, F], bf16, kind="ExternalInput")
    bv = nc.dram_tensor("b", [IN, 1], f32, kind="ExternalInput")  # tiled bias
    # Stacked output: out2[32*c + f, CH*b + e] = out[edge LD*b + CH*c + e, f].
    # Partitions 8..31 of each 32-group are garbage (never read by host).
    out2 = nc.dram_tensor("out2", [IN, NB * CH], bf16, kind="ExternalOutput")

    # Pipeline-counter semaphores (one per producer engine, computed
    # thresholds; DMA sems are per-queue, sound because each queue
    # completes its transfers in issue order):
    #   s_act: +1 per ACT activation   tanh[bb] -> 2bb+1, exp[j] -> 2j+2
    #   s_dve: +1 per DVE op           red[bb] -> 3bb+1, rcp[bb] -> 3bb+2,
    #                                  mul[j] -> 3j+3
    with ExitStack() as ctx:
        def mksem(name):
            return ctx.enter_context(nc.semaphore(name))

        s_in = [mksem("s_in_sp"), mksem("s_in_act")]
        s_out = mksem("s_out")
        s_mm = mksem("s_mm")
        s_act = mksem("s_act")
        s_dve = mksem("s_dve")
        s_const = mksem("s_const")

        in_t = [ctx.enter_context(nc.sbuf_tensor(f"in{r}", [IN, 3 * LD], f8))
                for r in range(NIN)]
        w_t = [ctx.enter_context(nc.sbuf_tensor(f"w{r}", [IN, CH], f32))
               for r in range(2)]
        ew_t = [ctx.enter_context(nc.sbuf_tensor(f"ew{r}", [IN, CH], bf16))
                for r in range(NEW)]
        o_t = [ctx.enter_context(nc.sbuf_tensor(f"o{r}", [IN, CH], bf16))
               for r in range(NWK)]
        dn_t = [ctx.enter_context(nc.sbuf_tensor(f"dn{r}", [IN, CH // DEG],
                                                 bf16)) for r in range(2)]
        rc_t = [ctx.enter_context(nc.sbuf_tensor(f"rc{r}", [IN, CH // DEG],
                                                 bf16)) for r in range(2)]
        ps_t = [ctx.enter_context(nc.psum_tensor(f"ps{r}", [IN, CH], f32))
                for r in range(NB)]
        w1_t = ctx.enter_context(nc.sbuf_tensor("w1s", [IN, F], bf16))
        w2_t = ctx.enter_context(nc.sbuf_tensor("w2s", [IN, F], bf16))
        b_t = ctx.enter_context(nc.sbuf_tensor("bs", [IN, 1], f32))

        def pk_rows(b):
            return pk[b * IN:(b + 1) * IN, :]

        with nc.Block() as block:

            @block.sync
            def _(sp):
                for b in range(0, NB, 2):
                    r = b % NIN
                    if b >= NIN:
                        sp.wait_ge(s_mm, NCT * (b - NIN + 1))
                    sp.dma_start(out=in_t[r][:], in_=pk_rows(b)
                                 ).then_inc(s_in[0], 16)

            @block.scalar
            def _(act):
                def act_exp(j):
                    # exp of bank j, one bank behind tanh: the gap gives the
                    # producer writes time to land after their sem fires
                    if SAFE_INTRA:
                        act.wait_ge(s_act, 2 * j + 1)
                    if j >= NEW:
                        act.wait_ge(s_dve, 3 * (j - NEW) + 3)
                    act.activation(
                        out=ew_t[j % NEW][:], in_=w_t[j % 2][:],
                        func=AF.Exp,
                    ).then_inc(s_act, 1)

                def act_bank(bb):
                    act.wait_ge(s_mm, NCT * (bb + 1))
                    if bb >= 1:
                        act_exp(bb - 1)
                    act.activation(
                        out=w_t[bb % 2][:], in_=ps_t[bb][:],
                        func=AF.Tanh, bias=b_t[:, 0:1],
                    ).then_inc(s_act, 1)

                LAG = 2
                for b in range(1, NB, 2):
                    r = b % NIN
                    if b >= NIN:
                        act.wait_ge(s_mm, NCT * (b - NIN + 1))
                    act.dma_start(out=in_t[r][:], in_=pk_rows(b)
                                  ).then_inc(s_in[1], 16)
                    if b == 1:
                        act.wait_ge(s_const, 48)
                    for bb in (b - LAG - 1, b - LAG):
                        if bb >= 0:
                            act_bank(bb)
                for bb in range(NB - LAG, NB):
                    act_bank(bb)
                act_exp(NB - 1)

            @block.tensor
            def _(pe):
                pe.wait_ge(s_const, 32)
                for b in range(NB):
                    r = b % NIN
                    pe.wait_ge(s_in[b % 2], 16 * (b // 2 + 1))
                    ps = ps_t[b]
                    it = in_t[r]
                    for c in range(NCT):
                        po = ps[32 * c:32 * c + F, :]
                        csl = slice(c * CH, (c + 1) * CH)
                        esl = slice(LD + c * CH, LD + (c + 1) * CH)
                        isl = slice(2 * LD + c * CH, 2 * LD + (c + 1) * CH)
                        pe.matmul(po, w1_t[:], it[:, csl],
                                  start=True, stop=False,
                                  tile_position=(0, 32 * c))
                        pe.matmul(po, w1_t[:], it[:, esl],
                                  start=False, stop=False,
                                  tile_position=(0, 32 * c))
                        # inc per col-tile: matmuls on different PE tiles can
                        # complete out of order, so the bank is only ready
                        # once ALL four closing matmuls have signalled
                        pe.matmul(po, w2_t[:], it[:, isl],
                                  start=False, stop=True,
                                  tile_position=(0, 32 * c)
                                  ).then_inc(s_mm, 1)

            @block.vector
            def _(dve):
                nseg = CH // DEG

                def dve_mul(j):
                    # mul of bank j, one bank behind reduce/recip: the gap
                    # gives producer writes time to land after their sem fires
                    if SAFE_INTRA:
                        dve.wait_ge(s_dve, 3 * j + 2)
                    if j >= NWK:
                        dve.wait_ge(s_out, 16 * (j - NWK + 1))
                    dve.tensor_mul(
                        out=o_t[j % NWK][:].rearrange("p (n d) -> p n d",
                                                      d=DEG),
                        in0=ew_t[j % NEW][:].rearrange("p (n d) -> p n d",
                                                       d=DEG),
                        in1=rc_t[j % 2][:].unsqueeze(-1).broadcast_to(
                            [IN, nseg, DEG]),
                    ).then_inc(s_dve, 1)

                for bb in range(NB):
                    dve.wait_ge(s_act, 2 * bb + 2)
                    if bb >= 1:
                        dve_mul(bb - 1)
                    with nc.allow_low_precision(
                            "32-wide segment sum of exp(tanh) in bf16; "
                            "error budget allows ~0.4%"):
                        dve.reduce_sum(
                            out=dn_t[bb % 2][:],
                            in_=ew_t[bb % NEW][:].rearrange(
                                "p (n d) -> p n d", d=DEG),
                            axis=mybir.AxisListType.X,
                        ).then_inc(s_dve, 1)
                        if SAFE_INTRA:
                            dve.wait_ge(s_dve, 3 * bb + 1)
                        dve.reciprocal(out=rc_t[bb % 2][:],
                                       in_=dn_t[bb % 2][:]
                                       ).then_inc(s_dve, 1)
                dve_mul(NB - 1)

            @block.gpsimd
            def _(gp):
                gp.dma_start(out=w1_t[:], in_=w1[:]).then_inc(s_const, 16)
                gp.dma_start(out=w2_t[:], in_=w2[:]).then_inc(s_const, 16)
                gp.dma_start(out=b_t[:], in_=bv[:]).then_inc(s_const, 16)
                for b in range(NB):
                    gp.wait_ge(s_dve, 3 * b + 3)
                    gp.dma_start(out=out2[:, b * CH:(b + 1) * CH],
                                 in_=o_t[b % NWK][:]
                                 ).then_inc(s_out, 16)
                gp.wait_ge(s_out, 16 * NB)

    nc.compile()
    return nc


# ---------------------------------------------------------------------------
# Fallback: previous all-f32r raw pipeline (measured ~105 us end-to-end).
# ---------------------------------------------------------------------------
LDR = 2048                 # raw-path input DMA batch (edges)
CHR = 512                  # raw-path matmul chunk


def _load_plan_raw():
    tail = [CHR, CHR, CHR, CHR // 2, CHR // 4, CHR // 4]
    loads = []
    pos = 0
    while pos < ES - sum(tail):
        loads.append((pos, LDR))
        pos += LDR
    for sz in tail:
        loads.append((pos, sz))
        pos += sz
    assert pos == ES, (pos, ES)
    return loads


def _build_bass_raw():
    import concourse.bacc as bacc
    from concourse import mybir

    f32 = mybir.dt.float32
    f32r = mybir.dt.float32r
    AF = mybir.ActivationFunctionType

    nc = bacc.Bacc("TRN2", target_bir_lowering=False, debug=False,
                   num_devices=N_CORES)

    xjT = nc.dram_tensor("xjT", [IN, ES], f32r, kind="ExternalInput")
    eijT = nc.dram_tensor("eijT", [IN, ES], f32r, kind="ExternalInput")
    xiT = nc.dram_tensor("xiT", [IN, ES], f32r, kind="ExternalInput")
    w1 = nc.dram_tensor("W1", [IN, F], f32r, kind="ExternalInput")
    w2 = nc.dram_tensor("W2", [IN, F], f32r, kind="ExternalInput")
    bv = nc.dram_tensor("b", [F, 1], f32, kind="ExternalInput")
    outT = nc.dram_tensor("outT", [F, ES], f32, kind="ExternalOutput")

    loads = _load_plan_raw()
    NB_ = len(loads)
    NIN_ = 5
    NWK_ = 3
    NEW_ = NWK_

    with ExitStack() as ctx:
        def mksem(name):
            return ctx.enter_context(nc.semaphore(name))

        s_xj = [mksem(f"s_xj{r}") for r in range(NIN_)]
        s_eij = [mksem(f"s_eij{r}") for r in range(NIN_)]
        s_xi = [mksem(f"s_xi{r}") for r in range(NIN_)]
        s_out = [mksem(f"s_out{r}") for r in range(NWK_)]
        s_mm = mksem("s_mm")
        s_red = mksem("s_red")
        s_rcp = mksem("s_rcp")
        s_psf = mksem("s_psf")
        s_exp = mksem("s_exp")
        s_mul = mksem("s_mul")
        s_const = mksem("s_const")

        in_xj = [ctx.enter_context(nc.sbuf_tensor(f"in_xj{r}", [IN, LDR], f32r))
                 for r in range(NIN_)]
        in_eij = [ctx.enter_context(nc.sbuf_tensor(f"in_eij{r}", [IN, LDR], f32r))
                  for r in range(NIN_)]
        in_xi = [ctx.enter_context(nc.sbuf_tensor(f"in_xi{r}", [IN, LDR], f32r))
                 for r in range(NIN_)]
        w_t = [ctx.enter_context(nc.sbuf_tensor(f"w{r}", [F, LDR], f32))
               for r in range(NWK_)]
        ew_t = [ctx.enter_context(nc.sbuf_tensor(f"ew{r}", [F, LDR], f32))
                for r in range(NEW_)]
        o_t = [ctx.enter_context(nc.sbuf_tensor(f"o{r}", [F, LDR], f32))
               for r in range(NWK_)]
        dn_t = ctx.enter_context(nc.sbuf_tensor("dn", [F, LDR // DEG], f32))
        rc_t = ctx.enter_context(nc.sbuf_tensor("rc", [F, LDR // DEG], f32))
        ps_t = [ctx.enter_context(nc.psum_tensor(f"ps{r}", [F, LDR], f32))
                for r in range(2)]
        w1_t = ctx.enter_context(nc.sbuf_tensor("w1s", [IN, F], f32r))
        w2_t = ctx.enter_context(nc.sbuf_tensor("w2s", [IN, F], f32r))
        b_t = ctx.enter_context(nc.sbuf_tensor("bs", [F, 1], f32))

        with nc.Block() as block:

            @block.sync
            def _(sp):
                for b, (pos, size) in enumerate(loads):
                    sl = slice(pos, pos + size)
                    if b >= NIN_:
                        sp.wait_ge(s_mm, b - (NIN_ - 1))
                    sp.dma_start(out=in_xj[b % NIN_][:, 0:size],
                                 in_=xjT[:, sl]).then_inc(s_xj[b % NIN_], 16)
                    if b % 2 == 0:
                        sp.dma_start(out=in_xi[b % NIN_][:, 0:size],
                                     in_=xiT[:, sl]).then_inc(s_xi[b % NIN_], 16)

            @block.scalar
            def _(act):
                def act_tail(bb):
                    bsz = loads[bb][1]
                    act.wait_ge(s_mm, bb + 1)
                    if bb >= NEW_:
                        act.wait_ge(s_mul, bb - (NEW_ - 1))
                    act.activation(
                        out=w_t[bb % NWK_][:, 0:bsz],
                        in_=ps_t[bb % 2][:, 0:bsz],
                        func=AF.Tanh, bias=b_t[:, 0:1],
                    ).then_inc(s_psf, 1)
                    if SAFE_INTRA:
                        act.wait_ge(s_psf, bb + 1)
                    act.activation(
                        out=ew_t[bb % NEW_][:, 0:bsz],
                        in_=w_t[bb % NWK_][:, 0:bsz],
                        func=AF.Exp,
                    ).then_inc(s_exp, 1)

                for b, (pos, size) in enumerate(loads):
                    sl = slice(pos, pos + size)
                    if b >= NIN_:
                        act.wait_ge(s_mm, b - (NIN_ - 1))
                    act.dma_start(out=in_eij[b % NIN_][:, 0:size],
                                  in_=eijT[:, sl]).then_inc(s_eij[b % NIN_], 16)
                    if b % 2 == 1:
                        act.dma_start(out=in_xi[b % NIN_][:, 0:size],
                                      in_=xiT[:, sl]).then_inc(s_xi[b % NIN_], 16)
                    if b >= 2:
                        bb = b - 2
                        if bb == 0:
                            act.wait_ge(s_const, 48)
                        act_tail(bb)
                for bb in (NB_ - 2, NB_ - 1):
                    act_tail(bb)

            @block.tensor
            def _(pe):
                pe.wait_ge(s_const, 48)
                for b, (pos, size) in enumerate(loads):
                    r = b % NIN_
                    n_use = b // NIN_ + 1
                    pe.wait_ge(s_xj[r], 16 * n_use)
                    pe.wait_ge(s_eij[r], 16 * n_use)
                    pe.wait_ge(s_xi[r], 16 * n_use)
                    if b >= 2:
                        pe.wait_ge(s_psf, b - 1)
                    ps = ps_t[b % 2]
                    nch = (size + CHR - 1) // CHR
                    for c in range(nch):
                        cw = min(CHR, size - c * CHR)
                        csl = slice(c * CHR, c * CHR + cw)
                        pe.matmul(ps[:, csl],
                                  w1_t[:], in_xj[b % NIN_][:, csl],
                                  start=True, stop=False)
                        pe.matmul(ps[:, csl],
                                  w1_t[:], in_eij[b % NIN_][:, csl],
                                  start=False, stop=False)
                        last = pe.matmul(ps[:, csl],
                                         w2_t[:], in_xi[b % NIN_][:, csl],
                                         start=False, stop=True)
                    last.then_inc(s_mm, 1)

            @block.vector
            def _(dve):
                for b, (pos, size) in enumerate(loads):
                    nseg = size // DEG
                    dve.wait_ge(s_exp, b + 1)
                    ew = ew_t[b % NEW_]
                    dve.reduce_sum(
                        out=dn_t[:, 0:nseg],
                        in_=ew[:, 0:size].rearrange("p (n d) -> p n d", d=DEG),
                        axis=mybir.AxisListType.X,
                    ).then_inc(s_red, 1)
                    if SAFE_INTRA:
                        dve.wait_ge(s_red, b + 1)
                    dve.reciprocal(
                        out=rc_t[:, 0:nseg], in_=dn_t[:, 0:nseg]
                    ).then_inc(s_rcp, 1)
                    if SAFE_INTRA:
                        dve.wait_ge(s_rcp, b + 1)
                    if b >= NWK_:
                        dve.wait_ge(s_out[b % NWK_], 16 * ((b - NWK_) // NWK_ + 1))
                    dve.tensor_mul(
                        out=o_t[b % NWK_][:, 0:size].rearrange(
                            "p (n d) -> p n d", d=DEG),
                        in0=ew[:, 0:size].rearrange("p (n d) -> p n d", d=DEG),
                        in1=rc_t[:, 0:nseg].unsqueeze(-1).broadcast_to(
                            [F, nseg, DEG]),
                    ).then_inc(s_mul, 1)

            @block.gpsimd
            def _(gp):
                gp.dma_start(out=w1_t[:], in_=w1[:]).then_inc(s_const, 16)
                gp.dma_start(out=w2_t[:], in_=w2[:]).then_inc(s_const, 16)
                gp.dma_start(out=b_t[:], in_=bv[:]).then_inc(s_const, 16)
                for b, (pos, size) in enumerate(loads):
                    sl = slice(pos, pos + size)
                    gp.wait_ge(s_mul, b + 1)
                    gp.dma_start(
                        out=outT[:, sl],
                        in_=o_t[b % NWK_][:, 0:size],
                    ).then_inc(s_out[b % NWK_], 16)
                for r in range(NWK_):
                    n_r = len(range(r, NB_, NWK_))
                    gp.wait_ge(s_out[r], 16 * n_r)

    nc.compile()
    return nc


def _get_compiled(mode):
    if mode not in _COMPILED:
        _COMPILED[mode] = _build_fp8() if mode == "fp8" else _build_bass_raw()
    return _COMPILED[mode]


def _prep_inputs_fp8(x_i, x_j, e_ij, W, b):
    import ml_dtypes

    F8 = ml_dtypes.float8_e3m4
    BF16 = ml_dtypes.bfloat16

    W = np.asarray(W, dtype=np.float32)
    W1 = np.ascontiguousarray(W[:IN]).astype(BF16)
    W2 = np.ascontiguousarray(W[IN:]).astype(BF16)
    bias = np.asarray(b, dtype=np.float32).reshape(F)
    btile = np.zeros((IN, 1), np.float32)
    for cc in range(NCT):
        btile[32 * cc:32 * cc + F, 0] = bias

    in_maps = []
    for c in range(N_CORES):
        sl = slice(c * ES, (c + 1) * ES)
        xjT = np.ascontiguousarray(np.asarray(x_j[sl], np.float32).T)
        eijT = np.ascontiguousarray(np.asarray(e_ij[sl], np.float32).T)
        xiT = np.ascontiguousarray(np.asarray(x_i[sl], np.float32).T)
        xj8 = xjT.astype(F8)
        # error feedback: fold xj's quantization error into eij before its
        # quantization, so q = xj + eij carries a single quantization error
        eij8 = (eijT + (xjT - xj8.astype(np.float32))).astype(F8)
        xi8 = xiT.astype(F8)
        # pack per load: contiguous [128, 3*LD] block b = [xj | eij | xi]
        pk = np.stack([xj8.reshape(IN, NB, LD),
                       eij8.reshape(IN, NB, LD),
                       xi8.reshape(IN, NB, LD)], axis=2)   # [IN, NB, 3, LD]
        pk = np.ascontiguousarray(pk.transpose(1, 0, 2, 3)
                                  ).reshape(NB * IN, 3 * LD)
        in_maps.append({
            "pk": pk,
            "W1": W1,
            "W2": W2,
            "b": btile,
        })
    return in_maps


def _gather_fp8(res):
    out = np.empty((E, F), dtype=np.float32)
    for c in range(N_CORES):
        o2 = np.asarray(res.results[c]["out2"]).astype(np.float32)
        o4 = o2.reshape(NCT, 32, NB, CH)[:, :F]              # [c, f, b, e]
        out[c * ES:(c + 1) * ES] = o4.transpose(2, 0, 3, 1).reshape(ES, F)
    return out


def _prep_inputs_raw(x_i, x_j, e_ij, W, b):
    W = np.ascontiguousarray(np.asarray(W, dtype=np.float32))
    bias = np.asarray(b, dtype=np.float32).reshape(F, 1)
    W1 = np.ascontiguousarray(W[:IN])
    W2 = np.ascontiguousarray(W[IN:])
    in_maps = []
    for c in range(N_CORES):
        sl = slice(c * ES, (c + 1) * ES)
        in_maps.append({
            "xjT": np.ascontiguousarray(np.asarray(x_j[sl]).T),
            "eijT": np.ascontiguousarray(np.asarray(e_ij[sl]).T),
            "xiT": np.ascontiguousarray(np.asarray(x_i[sl]).T),
            "W1": W1,
            "W2": W2,
            "b": bias,
        })
    return in_maps


def _gather_raw(res):
    out = np.empty((E, F), dtype=np.float32)
    for c in range(N_CORES):
        out[c * ES:(c + 1) * ES] = np.asarray(res.results[c]["outT"]).T
    return out


def _run_device(x_i, x_j, e_ij, W, b, trace=False, tmpdir=None,
                trace_cores=None, mode="fp8"):
    from concourse.bass_utils import run_bass_kernel_spmd

    nc = _get_compiled(mode)
    if mode == "fp8":
        in_maps = _prep_inputs_fp8(x_i, x_j, e_ij, W, b)
    else:
        in_maps = _prep_inputs_raw(x_i, x_j, e_ij, W, b)

    kwargs = {}
    if trace:
        kwargs.update(trace=True,
                      trace_cores=(trace_cores if trace_cores is not None
                                   else list(range(N_CORES))),
                      tmpdir=tmpdir)
    res = run_bass_kernel_spmd(nc, in_maps, core_ids=list(range(N_CORES)),
                               **kwargs)

    out = _gather_fp8(res) if mode == "fp8" else _gather_raw(res)
    return out, res


def _numpy_fallback(x_i, x_j, e_ij, adj, e_row, W, b):
    """Correct for arbitrary e_row (matches the reference semantics)."""
    x_i = np.asarray(x_i, np.float32)
    x_j = np.asarray(x_j, np.float32)
    e_ij = np.asarray(e_ij, np.float32)
    W = np.asarray(W, np.float32)
    b = np.asarray(b, np.float32)
    e_row = np.asarray(e_row).astype(np.int64)
    n = np.asarray(adj).shape[0]
    q = x_j + e_ij
    z = q @ W[:q.shape[1]] + x_i @ W[q.shape[1]:] + b
    w = np.tanh(z)
    m = np.full((n, w.shape[1]), -9e15, np.float32)
    np.maximum.at(m, e_row, w)
    ew = np.exp(w - m[e_row])
    denom = np.zeros((n, w.shape[1]), np.float32)
    np.add.at(denom, e_row, ew)
    return (ew / denom[e_row]).astype(np.float32)


def _is_fast_path(x_i, x_j, e_ij, adj, e_row, W, b):
    try:
        if np.asarray(x_i).shape != (E, IN):
            return False
        if np.asarray(x_j).shape != (E, IN):
            return False
        if np.asarray(e_ij).shape != (E, IN):
            return False
        if np.asarray(W).shape != (2 * IN, F):
            return False
        if np.asarray(b).reshape(-1).shape != (F,):
            return False
        if np.asarray(adj).shape[0] != N_NODES:
            return False
        er = np.asarray(e_row).reshape(-1)
        if er.shape != (E,):
            return False
        expected = np.repeat(np.arange(N_NODES, dtype=np.int64), DEG)
        return bool(np.array_equal(er.astype(np.int64), expected))
    except Exception:
        return False


def kernel(x_i, x_j, e_ij, adj, e_row, e_col, W, b, **_unused):
    if _is_fast_path(x_i, x_j, e_ij, adj, e_row, W, b):
        for mode in ("fp8", "raw"):
            try:
                out, _ = _run_device(x_i, x_j, e_ij, W, b, mode=mode)
                return out
            except Exception as e:  # fail safe: correct > fast
                print(f"kernel: device path '{mode}' failed "
                      f"({type(e).__name__}: {e}); trying next",
                      file=sys.stderr)
    return _numpy_fallback(x_i, x_j, e_ij, adj, e_row, W, b)
